# revision 13
# baseline (speedup 1.0000x reference)
"""Trainium2 Bass kernel for nn_GAT_Simple (2-layer GATConv with time-encoded
edge attrs, 50k nodes / 800k edges, 8 NeuronCores).

Sharding: nodes partitioned into 8 blocks; core k owns destination block k.
Per-core tables are node-ROTATED so each core's own block sits at rows [0,NB)
of its local tables, keeping the SPMD program core-independent.

Phases per core:
  Z0       - full (rotated) projected tables z0src = [x@W0 | alpha_src terms],
             z0dst = alpha_dst terms (dup-computed on every core; no comms).
  prephase - per-edge time-encoder attention scalars se = |cos(ts*w+b)| @ M
             for BOTH layers into a resident SBUF buffer (Sin ACT-table set
             stays out of the edge loop, which uses the Exp set).
  L0 edges - per 128-dst-node window: one multi-row indirect DMA gathers all
             ~2.2k edge source rows; alpha -> leaky-relu -> exp; weighted
             segment-sum AND softmax denominator via one one-hot matmul per
             128-edge tile accumulated in PSUM (normalize after aggregate).
             Window flush adds self-loops densely, normalizes, and fuses the
             layer-1 projection z1 = h@W1ext + c.
  AllGather of per-block z1 tables across the 8 cores.
  L1 edges - same structure on the gathered z1 tables; adds bias1; writes the
             per-core output block.
"""

import math
import os
from contextlib import ExitStack

import numpy as np

import concourse.bass as bass
import concourse.bacc as bacc
import concourse.tile as tile
from concourse import mybir
from concourse.bass_utils import run_bass_kernel_spmd

FP = mybir.dt.float32
I32 = mybir.dt.int32

N_CORES = 8
SLOPE = 0.2


class CFG:
    def __init__(self, N=50000, E=800000, IN=128, TD=64, H=4, C0=64, C1=16):
        self.N, self.E, self.IN, self.TD = N, E, IN, TD
        self.H, self.C0, self.C1 = H, C0, C1
        self.HC0, self.HC1 = H * C0, H * C1              # 256, 64
        self.D0 = self.HC0 + 4                            # z0 | as0  -> 260
        self.D1 = self.HC1 + 4                            # z1 | as1  -> 68
        self.NP = ((N + N_CORES * 128 - 1) // (N_CORES * 128)) * (N_CORES * 128)
        self.NB = self.NP // N_CORES
        self.W = self.NB // 128


# ---------------------------------------------------------------- host prep

def _host_prep(cfg, x, edge_index, timestamps, time_w, time_b,
               W0, att_src0, att_dst0, lin_edge0, att_edge0, bias0,
               W1, att_src1, att_dst1, lin_edge1, att_edge1, bias1):
    N, NP, NB, W = cfg.N, cfg.NP, cfg.NB, cfg.W
    H, C0, C1, TD, IN = cfg.H, cfg.C0, cfg.C1, cfg.TD, cfg.IN
    HC0, HC1 = cfg.HC0, cfg.HC1

    f32 = np.float32
    x = np.asarray(x, f32)
    src = np.asarray(edge_index[0], np.int64)
    dst = np.asarray(edge_index[1], np.int64)
    ts = np.asarray(timestamps, f32)

    def blockdiag(att, C):  # [H, C] -> [H*C, H]
        out = np.zeros((H * C, H), f32)
        for h in range(H):
            out[h * C:(h + 1) * C, h] = att[h]
        return out

    W0 = np.asarray(W0, f32); W1 = np.asarray(W1, f32)
    W0ext = np.concatenate(
        [W0, W0 @ blockdiag(np.asarray(att_src0, f32), C0),
         W0 @ blockdiag(np.asarray(att_dst0, f32), C0)], axis=1)  # [IN, HC0+8]
    W1ext = np.concatenate(
        [W1, W1 @ blockdiag(np.asarray(att_src1, f32), C1),
         W1 @ blockdiag(np.asarray(att_dst1, f32), C1)], axis=1)  # [HC0, HC1+8]
    c1ext = np.asarray(bias0, f32) @ W1ext                         # [HC1+8]

    M0c = np.einsum("thc,hc->th",
                    np.asarray(lin_edge0, f32).reshape(TD, H, C0),
                    np.asarray(att_edge0, f32))
    M1c = np.einsum("thc,hc->th",
                    np.asarray(lin_edge1, f32).reshape(TD, H, C1),
                    np.asarray(att_edge1, f32))
    M01 = np.concatenate([M0c, M1c], axis=1).astype(f32)           # [TD, 2H]
    M01x2 = np.concatenate([M01, M01], axis=0)                     # [2*TD, 2H]
    sl0 = M0c.sum(axis=0).astype(f32)
    sl1 = M1c.sum(axis=0).astype(f32)

    # |cos(ts*w+b)| = |sin(2*pi*m - pi)|, m = frac(ts*(w/2pi) + (b+pi/2)/2pi + .5)
    w2pi = (np.asarray(time_w, f32) / (2 * np.pi)).astype(f32)
    b2pi = ((np.asarray(time_b, f32) + np.pi / 2) / (2 * np.pi)).astype(f32)
    efsc = np.tile(w2pi, 2).reshape(128, 1)
    efbi = np.tile(b2pi, 2).reshape(128, 1)

    ones2 = np.zeros((2, 128), f32)
    ones2[0, :64] = 1.0
    ones2[1, 64:] = 1.0

    iota_row = np.broadcast_to(np.arange(128, dtype=f32), (128, 128)).copy()
    identity = np.eye(128, dtype=f32)

    xpadT = np.zeros((IN, NP), f32)
    xpadT[:, :N] = x.T

    blk = dst // NB
    per_core = []
    for k in range(N_CORES):
        m = blk == k
        s_k, d_k, t_k = src[m], dst[m], ts[m]
        order = np.argsort(d_k, kind="stable")
        per_core.append((s_k[order], d_k[order], t_k[order]))

    cnt = np.zeros((N_CORES, W), np.int64)
    for k in range(N_CORES):
        d_k = per_core[k][1] - k * NB
        cnt[k] = np.bincount(d_k // 128, minlength=W)
    Tw = np.maximum(1, (cnt.max(axis=0) + 127) // 128).astype(np.int64)
    CT = int(Tw.sum())
    CTpre = ((CT + 7) // 8) * 8
    colbase = np.zeros(W + 1, np.int64)
    colbase[1:] = np.cumsum(Tw)

    consts = {
        "W0ext": W0ext, "W1extA": W1ext[:128].copy(), "W1extB": W1ext[128:].copy(),
        "M01": M01x2, "efsc": efsc, "efbi": efbi, "ones2": ones2,
        "iota": iota_row, "ident": identity,
        "sl0": np.broadcast_to(sl0, (128, H)).copy(),
        "sl1": np.broadcast_to(sl1, (128, H)).copy(),
        "bias1": np.broadcast_to(np.asarray(bias1, f32), (128, HC1)).copy(),
        "c1ext": np.broadcast_to(c1ext, (128, HC1 + 8)).copy(),
        "negpi": np.full((128, 1), -np.pi, f32),
    }

    in_maps = []
    for k in range(N_CORES):
        s_k, d_k, t_k = per_core[k]
        esrc0 = np.zeros((128, CT), np.int32)
        esrc1 = np.zeros((128, CT), np.int32)
        eslot = np.zeros((128, CT), f32)
        evalid = np.zeros((128, CT * H), f32)
        ets = np.zeros(CTpre * 128, f32)

        pos = 0
        for w in range(W):
            n = int(cnt[k, w])
            sw = s_k[pos:pos + n]; dw = d_k[pos:pos + n]; tw = t_k[pos:pos + n]
            pos += n
            G = int(Tw[w]); c0 = int(colbase[w])
            sl = (dw - k * NB - w * 128).astype(f32)
            for g in range(G):
                lo = g * 128
                m = min((g + 1) * 128, n) - lo
                if m <= 0:
                    continue
                c = c0 + g
                esrc0[:m, c] = ((sw[lo:lo + m] - k * NB) % NP).astype(np.int32)
                esrc1[:m, c] = sw[lo:lo + m].astype(np.int32)
                eslot[:m, c] = sl[lo:lo + m]
                evalid[:m, c * H:(c + 1) * H] = 1.0
                ets[c * 128:c * 128 + m] = tw[lo:lo + m]
        im = {
            "xT": np.roll(xpadT, -k * NB, axis=1).copy(),
            "esrc0": esrc0, "esrc1": esrc1,
            "eslot": eslot, "evalid": evalid, "ets": ets,
        }
        im.update(consts)
        in_maps.append(im)

    meta = {"Tw": Tw, "colbase": colbase, "CT": CT, "CTpre": CTpre}
    return in_maps, meta


# ---------------------------------------------------------------- program

def _build_program(cfg, meta):
    NP, NB, W = cfg.NP, cfg.NB, cfg.W
    H, TD, IN = cfg.H, cfg.TD, cfg.IN
    HC0, HC1, D0, D1 = cfg.HC0, cfg.HC1, cfg.D0, cfg.D1
    Tw, colbase, CT, CTpre = meta["Tw"], meta["colbase"], meta["CT"], meta["CTpre"]

    nc = bacc.Bacc("TRN2", target_bir_lowering=False, debug=False,
                   num_devices=N_CORES)
    P = nc.declare_dram_parameter

    xT = P("xT", [IN, NP], FP, isOutput=False)
    esrc0 = P("esrc0", [128, CT], I32, isOutput=False)
    esrc1 = P("esrc1", [128, CT], I32, isOutput=False)
    eslot = P("eslot", [128, CT], FP, isOutput=False)
    evalid = P("evalid", [128, CT * H], FP, isOutput=False)
    ets = P("ets", [CTpre * 128], FP, isOutput=False)
    W0ext_d = P("W0ext", [IN, HC0 + 8], FP, isOutput=False)
    W1A_d = P("W1extA", [128, HC1 + 8], FP, isOutput=False)
    W1B_d = P("W1extB", [128, HC1 + 8], FP, isOutput=False)
    M01_d = P("M01", [2 * TD, 2 * H], FP, isOutput=False)
    efsc_d = P("efsc", [128, 1], FP, isOutput=False)
    efbi_d = P("efbi", [128, 1], FP, isOutput=False)
    ones2_d = P("ones2", [2, 128], FP, isOutput=False)
    iota_d = P("iota", [128, 128], FP, isOutput=False)
    ident_d = P("ident", [128, 128], FP, isOutput=False)
    sl0_d = P("sl0", [128, H], FP, isOutput=False)
    sl1_d = P("sl1", [128, H], FP, isOutput=False)
    bias1_d = P("bias1", [128, HC1], FP, isOutput=False)
    c1ext_d = P("c1ext", [128, HC1 + 8], FP, isOutput=False)
    negpi_d = P("negpi", [128, 1], FP, isOutput=False)

    out_d = P("out", [NB, HC1], FP, isOutput=True)
    KDEBUG = int(os.environ.get("KDEBUG", "0"))
    if KDEBUG:
        dz0_d = P("dz0", [NP, D0], FP, isOutput=True)
        dse_d = P("dse", [128, CTpre * 2 * H], FP, isOutput=True)
        dz1_d = P("dz1", [NB, D1], FP, isOutput=True)

    z0src = nc.dram_tensor("z0src", [NP, D0], FP)
    z0dst = nc.dram_tensor("z0dst", [NB, H], FP)
    z1loc = nc.dram_tensor("z1loc", [NB, D1], FP)
    z1dloc = nc.dram_tensor("z1dloc", [NB, H], FP)
    z1src_g = nc.dram_tensor("z1src_g", [NP, D1], FP, addr_space="Shared")

    AF = mybir.ActivationFunctionType
    OP = mybir.AluOpType
    TWO_PI = 2 * math.pi

    with ExitStack() as ctx:
        tc = ctx.enter_context(tile.TileContext(nc))
        cpool = ctx.enter_context(tc.tile_pool(name="consts", bufs=1))
        sepool = ctx.enter_context(tc.tile_pool(name="seall", bufs=1))
        sbuf = ctx.enter_context(tc.tile_pool(name="sbuf", bufs=3))
        gpool = ctx.enter_context(tc.tile_pool(name="gather", bufs=2))
        rpool = ctx.enter_context(tc.tile_pool(name="rhs", bufs=3))
        mpool = ctx.enter_context(tc.tile_pool(name="onehot", bufs=int(max(Tw)) + 2))
        mtpool = ctx.enter_context(tc.tile_pool(name="onehot_t", bufs=2))
        fpool = ctx.enter_context(tc.tile_pool(name="flush", bufs=2))
        ppre = ctx.enter_context(tc.tile_pool(name="ppre", bufs=2, space="PSUM"))
        pse = ctx.enter_context(tc.tile_pool(name="pse", bufs=1, space="PSUM"))
        pseg = ctx.enter_context(tc.tile_pool(name="pseg", bufs=2, space="PSUM"))
        ptr = ctx.enter_context(tc.tile_pool(name="ptr", bufs=1, space="PSUM"))
        pz1 = ctx.enter_context(tc.tile_pool(name="pz1", bufs=1, space="PSUM"))

        def cload(dram, shape, tag):
            t = cpool.tile(shape, FP, tag=tag)
            nc.sync.dma_start(out=t[:], in_=dram[:])
            return t

        W0ext_s = cload(W0ext_d, [IN, HC0 + 8], "w0ext")
        W1A_s = cload(W1A_d, [128, HC1 + 8], "w1a")
        W1B_s = cload(W1B_d, [128, HC1 + 8], "w1b")
        M01_s = cload(M01_d, [2 * TD, 2 * H], "m01")
        efsc_s = cload(efsc_d, [128, 1], "efsc")
        efbi_s = cload(efbi_d, [128, 1], "efbi")
        ones2_s = cload(ones2_d, [2, 128], "ones2")
        iota_s = cload(iota_d, [128, 128], "iota")
        ident_s = cload(ident_d, [128, 128], "ident")
        sl0_s = cload(sl0_d, [128, H], "sl0")
        sl1_s = cload(sl1_d, [128, H], "sl1")
        bias1_s = cload(bias1_d, [128, HC1], "bias1")
        c1ext_s = cload(c1ext_d, [128, HC1 + 8], "c1ext")
        negpi_s = cload(negpi_d, [128, 1], "negpi")

        se_all = sepool.tile([128, CTpre * 2 * H], FP)

        # ---- Phase Z0
        for t in range(NP // 128):
            xt = sbuf.tile([IN, 128], FP, tag="xt")
            nc.sync.dma_start(out=xt[:], in_=xT[:, t * 128:(t + 1) * 128])
            ps = pseg.tile([128, HC0 + 8], FP, tag="segp")
            nc.tensor.matmul(out=ps[:], lhsT=xt[:], rhs=W0ext_s[:],
                             start=True, stop=True)
            zt = sbuf.tile([128, HC0 + 8], FP, tag="zt")
            nc.vector.tensor_copy(out=zt[:], in_=ps[:])
            nc.sync.dma_start(out=z0src[t * 128:(t + 1) * 128, :],
                              in_=zt[:, :D0])
            if t < W:
                nc.sync.dma_start(out=z0dst[t * 128:(t + 1) * 128, :],
                                  in_=zt[:, D0:D0 + H])

        PH0 = int(os.environ.get("KPHASES", "4"))
        # ---- Prephase
        ets_v = ets[:].rearrange("(a b) -> a b", b=512)
        for c in (range(0, CTpre, 8) if PH0 >= 2 else []):
            ts2 = sbuf.tile([2, 512], FP, tag="ts2")
            row = c * 128 // 512
            nc.sync.dma_start(out=ts2[:], in_=ets_v[row:row + 2, :])
            rp = ppre.tile([128, 512], FP, tag="prep")
            nc.tensor.matmul(out=rp[:], lhsT=ones2_s[:], rhs=ts2[:],
                             start=True, stop=True)
            t1 = sbuf.tile([128, 512], FP, tag="t1")
            nc.scalar.activation(out=t1[:], in_=rp[:], func=AF.Identity,
                                 bias=efbi_s[:], scale=efsc_s[:])
            k32 = sbuf.tile([128, 512], I32, tag="k32")
            nc.vector.tensor_copy(out=k32[:], in_=t1[:])
            kf = sbuf.tile([128, 512], FP, tag="kf")
            nc.vector.tensor_copy(out=kf[:], in_=k32[:])
            nc.vector.tensor_tensor(out=t1[:], in0=t1[:], in1=kf[:],
                                    op=OP.subtract)
            nc.scalar.activation(out=t1[:], in_=t1[:], func=AF.Sin,
                                 scale=TWO_PI)
            nc.scalar.activation(out=t1[:], in_=t1[:], func=AF.Abs)
            for b in range(8):
                half, coff = (b // 4) * 64, (b % 4) * 128
                sp = pse.tile([128, 2 * H], FP, tag="sep")
                nc.tensor.matmul(out=sp[:],
                                 lhsT=t1[half:half + 64, coff:coff + 128],
                                 rhs=M01_s[half:half + 64, :],
                                 start=True, stop=True)
                nc.vector.tensor_copy(
                    out=se_all[:, (c + b) * 2 * H:(c + b + 1) * 2 * H],
                    in_=sp[:])

        # ---- edge pass (layer 0 / 1)
        def edge_pass(layer):
            if layer == 0:
                table_s, dense_s, dense_d = z0src, z0src, z0dst
                esrc = esrc0
                DS, HC, seoff = D0, HC0, 0
            else:
                table_s, dense_s, dense_d = z1src_g, z1loc, z1dloc
                esrc = esrc1
                DS, HC, seoff = D1, HC1, H
            Cc = HC // H
            for w in range(W):
                G = int(Tw[w]); c0 = int(colbase[w])
                r0 = w * 128
                isrc = sbuf.tile([128, G], I32, tag="isrc")
                nc.sync.dma_start(out=isrc[:], in_=esrc[:, c0:c0 + G])
                slotf = sbuf.tile([128, G], FP, tag="slotf")
                nc.sync.dma_start(out=slotf[:], in_=eslot[:, c0:c0 + G])
                evw = sbuf.tile([128, G * H], FP, tag="evw")
                nc.sync.dma_start(out=evw[:],
                                  in_=evalid[:, c0 * H:(c0 + G) * H])
                # dense own-window rows (self loops + ad_w)
                zw = fpool.tile([128, DS], FP, tag=f"zw{layer}")
                nc.sync.dma_start(out=zw[:], in_=dense_s[r0:r0 + 128, :])
                adw = fpool.tile([128, H], FP, tag="adw")
                nc.sync.dma_start(out=adw[:], in_=dense_d[r0:r0 + 128, :])

                zg = gpool.tile([128, G * DS], FP, tag=f"zg{layer}")
                Ms = []
                for g in range(G):
                    nc.gpsimd.indirect_dma_start(
                        out=zg[:, g * DS:(g + 1) * DS], out_offset=None,
                        in_=table_s[:],
                        in_offset=bass.IndirectOffsetOnAxis(
                            ap=isrc[:, g:g + 1], axis=0))
                    M = mpool.tile([128, 128], FP, tag="M")
                    nc.vector.tensor_tensor(
                        out=M[:],
                        in0=slotf[:, g:g + 1].to_broadcast([128, 128]),
                        in1=iota_s[:], op=OP.is_equal)
                    Ms.append(M)

                # ad[dst] per edge via transposed one-hot: adexp[:, g*H:] =
                # Mt.T @ ad_w with Mt = M^T
                adexp = sbuf.tile([128, G * H], FP, tag="adexp")
                for g in range(G):
                    tp = ptr.tile([128, 128], FP, tag="tp")
                    nc.tensor.transpose(out=tp[:], in_=Ms[g][:],
                                        identity=ident_s[:])
                    Mt = mtpool.tile([128, 128], FP, tag="Mt")
                    nc.vector.tensor_copy(out=Mt[:], in_=tp[:])
                    adp = pse.tile([128, H], FP, tag="adp")
                    nc.tensor.matmul(out=adp[:], lhsT=Mt[:], rhs=adw[:],
                                     start=True, stop=True)
                    nc.vector.tensor_copy(out=adexp[:, g * H:(g + 1) * H],
                                          in_=adp[:])

                zg_v = zg[:].rearrange("p (g d) -> p g d", d=DS)
                se_v = se_all[:, c0 * 2 * H:(c0 + G) * 2 * H].rearrange(
                    "p (g e) -> p g e", e=2 * H)

                # alpha = se + as[src] + ad[dst]; lrelu; exp; mask padding
                alpha = sbuf.tile([128, G * H], FP, tag="alpha")
                al_v = alpha[:].rearrange("p (g h) -> p g h", h=H)
                nc.vector.tensor_tensor(
                    out=al_v, in0=se_v[:, :, seoff:seoff + H],
                    in1=zg_v[:, :, HC:HC + H], op=OP.add)
                nc.vector.tensor_tensor(
                    out=alpha[:], in0=alpha[:], in1=adexp[:], op=OP.add)
                scaled = sbuf.tile([128, G * H], FP, tag="scaled")
                nc.vector.tensor_scalar(out=scaled[:], in0=alpha[:],
                                        scalar1=SLOPE, scalar2=None,
                                        op0=OP.mult)
                nc.vector.tensor_tensor(out=alpha[:], in0=alpha[:],
                                        in1=scaled[:], op=OP.max)
                expw = sbuf.tile([128, G * H], FP, tag="expw")
                nc.scalar.activation(out=expw[:], in_=alpha[:], func=AF.Exp)
                nc.vector.tensor_tensor(out=expw[:], in0=expw[:],
                                        in1=evw[:], op=OP.mult)

                ps = pseg.tile([128, HC + H], FP, tag="segp")
                for g in range(G):
                    rhs = rpool.tile([128, HC + H], FP, tag=f"rhs{layer}")
                    for h in range(H):
                        nc.vector.tensor_scalar(
                            out=rhs[:, h * Cc:(h + 1) * Cc],
                            in0=zg_v[:, g, h * Cc:(h + 1) * Cc],
                            scalar1=expw[:, g * H + h:g * H + h + 1],
                            scalar2=None, op0=OP.mult)
                    nc.vector.tensor_copy(out=rhs[:, HC:HC + H],
                                          in_=expw[:, g * H:(g + 1) * H])
                    nc.tensor.matmul(out=ps[:], lhsT=Ms[g][:], rhs=rhs[:],
                                     start=(g == 0), stop=(g == G - 1))

                # ---- flush
                sl_s = sl0_s if layer == 0 else sl1_s
                asel = fpool.tile([128, H], FP, tag="asel")
                nc.vector.tensor_tensor(out=asel[:], in0=zw[:, HC:HC + H],
                                        in1=adw[:], op=OP.add)
                nc.vector.tensor_tensor(out=asel[:], in0=asel[:],
                                        in1=sl_s[:], op=OP.add)
                ssc = fpool.tile([128, H], FP, tag="ssc")
                nc.vector.tensor_scalar(out=ssc[:], in0=asel[:], scalar1=SLOPE,
                                        scalar2=None, op0=OP.mult)
                nc.vector.tensor_tensor(out=asel[:], in0=asel[:], in1=ssc[:],
                                        op=OP.max)
                nc.scalar.activation(out=asel[:], in_=asel[:], func=AF.Exp)
                den = fpool.tile([128, H], FP, tag="den")
                nc.vector.tensor_tensor(out=den[:], in0=ps[:, HC:HC + H],
                                        in1=asel[:], op=OP.add)
                rec = fpool.tile([128, H], FP, tag="rec")
                nc.vector.reciprocal(out=rec[:], in_=den[:])
                o0 = fpool.tile([128, HC], FP, tag=f"o0{layer}")
                for h in range(H):
                    hs = slice(h * Cc, (h + 1) * Cc)
                    nc.vector.tensor_scalar(
                        out=o0[:, hs], in0=zw[:, hs],
                        scalar1=asel[:, h:h + 1], scalar2=None, op0=OP.mult)
                    nc.vector.tensor_tensor(out=o0[:, hs], in0=o0[:, hs],
                                            in1=ps[:, hs], op=OP.add)
                    nc.vector.tensor_scalar(
                        out=o0[:, hs], in0=o0[:, hs],
                        scalar1=rec[:, h:h + 1], scalar2=None, op0=OP.mult)

                if layer == 0:
                    z1p = pz1.tile([128, HC1 + 8], FP, tag="z1p")
                    for half, Wh in ((0, W1A_s), (1, W1B_s)):
                        tp = ptr.tile([128, 128], FP, tag="tp")
                        nc.tensor.transpose(
                            out=tp[:], in_=o0[:, half * 128:(half + 1) * 128],
                            identity=ident_s[:])
                        oT = fpool.tile([128, 128], FP, tag="oT")
                        nc.vector.tensor_copy(out=oT[:], in_=tp[:])
                        nc.tensor.matmul(out=z1p[:], lhsT=oT[:], rhs=Wh[:],
                                         start=(half == 0), stop=(half == 1))
                    z1e = fpool.tile([128, HC1 + 8], FP, tag="z1e")
                    nc.vector.tensor_tensor(out=z1e[:], in0=z1p[:],
                                            in1=c1ext_s[:], op=OP.add)
                    nc.sync.dma_start(out=z1loc[r0:r0 + 128, :],
                                      in_=z1e[:, :D1])
                    nc.sync.dma_start(out=z1dloc[r0:r0 + 128, :],
                                      in_=z1e[:, D1:D1 + H])
                else:
                    nc.vector.tensor_tensor(out=o0[:], in0=o0[:],
                                            in1=bias1_s[:], op=OP.add)
                    nc.sync.dma_start(out=out_d[r0:r0 + 128, :], in_=o0[:])

        PH = PH0
        if PH >= 3:
            edge_pass(0)
        if KDEBUG:
            nc.sync.dma_start(out=dz0_d[:], in_=z0src[:])
            nc.sync.dma_start(out=dse_d[:], in_=se_all[:])
            if PH >= 3:
                nc.sync.dma_start(out=dz1_d[:], in_=z1loc[:])

        if PH >= 4:
            nc.gpsimd.collective_compute(
                "AllGather", mybir.AluOpType.bypass,
                replica_groups=[list(range(N_CORES))],
                ins=[z1loc[:]], outs=[z1src_g[:]])

            edge_pass(1)

    nc.compile()
    return nc


# ---------------------------------------------------------------- entry

def kernel(**inputs):
    cfg = CFG(N=inputs["x"].shape[0], E=inputs["edge_index"].shape[1],
              IN=inputs["x"].shape[1], TD=inputs["time_w"].shape[0],
              H=np.asarray(inputs["att_src0"]).shape[0],
              C0=np.asarray(inputs["att_src0"]).shape[1],
              C1=np.asarray(inputs["att_src1"]).shape[1])
    in_maps, meta = _host_prep(cfg, **{k: np.asarray(v) for k, v in inputs.items()})
    nc = _build_program(cfg, meta)
    res = run_bass_kernel_spmd(nc, in_maps, list(range(N_CORES)))
    blocks = [res.results[k]["out"] for k in range(N_CORES)]
    return np.concatenate(blocks, axis=0)[:cfg.N].astype(np.float32)


# revision 18
# speedup vs baseline: 1.1837x; 1.1837x over previous
"""Trainium2 Bass kernel for nn_GAT_Simple (2-layer GATConv with time-encoded
edge attrs, 50k nodes / 800k edges, 8 NeuronCores).

Sharding: nodes partitioned into 8 blocks; core k owns destination block k.
Per-core tables are node-ROTATED so each core's own block sits at rows [0,NB)
of its local tables, keeping the SPMD program core-independent.

Phases per core:
  Z0       - full (rotated) projected tables z0src = bf16 [x@W0 | alpha_src
             terms], z0dst = fp32 alpha_dst terms for the local block only.
  prephase - per-edge time-encoder attention scalars se = |cos(ts*w+b)| @ M
             for BOTH layers into a resident SBUF buffer (keeps the Sin ACT
             table set out of the edge loop, which uses the Exp set).
  L0 edges - per 128-dst-node window: per-tile 128-row indirect gathers of
             bf16 source rows; alpha -> leaky-relu -> exp; ad[dst] expanded
             from a dense per-window vector via a transposed one-hot matmul
             accumulated in PSUM; weighted segment-sum + softmax denominator
             via a one-hot matmul per tile accumulated in PSUM (normalize
             after aggregation => single pass over edges). Window flush adds
             self-loops densely, normalizes, and fuses the layer-1 projection.
  AllGather of per-block bf16 z1 tables across the 8 cores.
  L1 edges - same structure on the gathered z1 tables; adds bias1; writes the
             per-core fp32 output block.
"""

import math
import os
from contextlib import ExitStack

import numpy as np

try:
    import ml_dtypes  # noqa: F401  (registers bfloat16 with numpy)
    _BF16 = np.dtype("bfloat16")
except ImportError:
    _BF16 = None

import concourse.bass as bass
import concourse.bacc as bacc
import concourse.tile as tile
from concourse import mybir
from concourse.bass_utils import run_bass_kernel_spmd

FP = mybir.dt.float32
BF = mybir.dt.bfloat16
I32 = mybir.dt.int32

N_CORES = 8
SLOPE = 0.2


class CFG:
    def __init__(self, N=50000, E=800000, IN=128, TD=64, H=4, C0=64, C1=16):
        self.N, self.E, self.IN, self.TD = N, E, IN, TD
        self.H, self.C0, self.C1 = H, C0, C1
        self.HC0, self.HC1 = H * C0, H * C1              # 256, 64
        self.D0 = self.HC0 + 4                            # z0 | as0  -> 260
        self.D1 = self.HC1 + 4                            # z1 | as1  -> 68
        self.NP = ((N + N_CORES * 128 - 1) // (N_CORES * 128)) * (N_CORES * 128)
        self.NB = self.NP // N_CORES
        self.W = self.NB // 128


# ---------------------------------------------------------------- host prep

def _host_prep(cfg, x, edge_index, timestamps, time_w, time_b,
               W0, att_src0, att_dst0, lin_edge0, att_edge0, bias0,
               W1, att_src1, att_dst1, lin_edge1, att_edge1, bias1):
    N, NP, NB, W = cfg.N, cfg.NP, cfg.NB, cfg.W
    H, C0, C1, TD, IN = cfg.H, cfg.C0, cfg.C1, cfg.TD, cfg.IN
    HC0, HC1 = cfg.HC0, cfg.HC1

    f32 = np.float32
    bf = _BF16 if int(os.environ.get("KBF16", "1")) else f32
    x = np.asarray(x, f32)
    src = np.asarray(edge_index[0], np.int64)
    dst = np.asarray(edge_index[1], np.int64)
    ts = np.asarray(timestamps, f32)

    def blockdiag(att, C):  # [H, C] -> [H*C, H]
        out = np.zeros((H * C, H), f32)
        for h in range(H):
            out[h * C:(h + 1) * C, h] = att[h]
        return out

    W0 = np.asarray(W0, f32); W1 = np.asarray(W1, f32)
    W0ext = np.concatenate(
        [W0, W0 @ blockdiag(np.asarray(att_src0, f32), C0),
         W0 @ blockdiag(np.asarray(att_dst0, f32), C0)], axis=1)  # [IN, HC0+8]
    W1ext = np.concatenate(
        [W1, W1 @ blockdiag(np.asarray(att_src1, f32), C1),
         W1 @ blockdiag(np.asarray(att_dst1, f32), C1)], axis=1)  # [HC0, HC1+8]
    c1ext = np.asarray(bias0, f32) @ W1ext                         # [HC1+8]

    M0c = np.einsum("thc,hc->th",
                    np.asarray(lin_edge0, f32).reshape(TD, H, C0),
                    np.asarray(att_edge0, f32))
    M1c = np.einsum("thc,hc->th",
                    np.asarray(lin_edge1, f32).reshape(TD, H, C1),
                    np.asarray(att_edge1, f32))
    M01 = np.concatenate([M0c, M1c], axis=1).astype(f32)           # [TD, 2H]
    M01x2 = np.concatenate([M01, M01], axis=0)                     # [2TD, 2H]
    sl0 = M0c.sum(axis=0).astype(f32)
    sl1 = M1c.sum(axis=0).astype(f32)

    # |cos(ts*w+b)| = |sin(2*pi*r)|, r = t1 - round(t1),
    # t1 = ts*(w/2pi) + (b+pi/2)/(2pi)
    w2pi = (np.asarray(time_w, f32) / (2 * np.pi)).astype(f32)
    b2pi = ((np.asarray(time_b, f32) + np.pi / 2) / (2 * np.pi)).astype(f32)
    efsc = np.tile(w2pi, 2).reshape(128, 1)
    efbi = np.tile(b2pi, 2).reshape(128, 1)

    ones2 = np.zeros((2, 128), f32)
    ones2[0, :64] = 1.0
    ones2[1, 64:] = 1.0

    iota_row = np.broadcast_to(np.arange(128, dtype=f32), (128, 128)).copy()
    iota_col = np.broadcast_to(np.arange(128, dtype=f32)[:, None],
                               (128, 128)).copy()
    identity_bf = np.eye(128).astype(bf)

    xpadT = np.zeros((IN, NP), f32)
    xpadT[:, :N] = x.T

    blk = dst // NB
    per_core = []
    for k in range(N_CORES):
        m = blk == k
        s_k, d_k, t_k = src[m], dst[m], ts[m]
        order = np.argsort(d_k, kind="stable")
        per_core.append((s_k[order], d_k[order], t_k[order]))

    cnt = np.zeros((N_CORES, W), np.int64)
    for k in range(N_CORES):
        d_k = per_core[k][1] - k * NB
        cnt[k] = np.bincount(d_k // 128, minlength=W)
    Tw = np.maximum(1, (cnt.max(axis=0) + 127) // 128).astype(np.int64)
    CT = int(Tw.sum())
    CTpre = ((CT + 7) // 8) * 8
    colbase = np.zeros(W + 1, np.int64)
    colbase[1:] = np.cumsum(Tw)

    consts = {
        "W0ext": W0ext.astype(bf),
        "W1extA": W1ext[:128].astype(bf), "W1extB": W1ext[128:].astype(bf),
        "M01": M01x2, "efsc": efsc, "efbi": efbi, "ones2": ones2,
        "iota": iota_row, "iotac": iota_col, "ident": identity_bf,
        "sl0": np.broadcast_to(sl0, (128, H)).copy(),
        "sl1": np.broadcast_to(sl1, (128, H)).copy(),
        "bias1": np.broadcast_to(np.asarray(bias1, f32), (128, HC1)).copy(),
        "c1ext": np.broadcast_to(c1ext, (128, HC1 + 8)).copy(),
    }

    in_maps = []
    for k in range(N_CORES):
        s_k, d_k, t_k = per_core[k]
        esrc0 = np.zeros((128, CT), np.int32)
        esrc1 = np.zeros((128, CT), np.int32)
        eslot = np.zeros((128, CT), f32)
        eslotT = np.zeros(CT * 128, bf)
        evalid = np.zeros((128, CT * H), f32)
        ets = np.zeros(CTpre * 128, f32)

        pos = 0
        for w in range(W):
            n = int(cnt[k, w])
            sw = s_k[pos:pos + n]; dw = d_k[pos:pos + n]; tw = t_k[pos:pos + n]
            pos += n
            G = int(Tw[w]); c0 = int(colbase[w])
            sl = (dw - k * NB - w * 128).astype(f32)
            for g in range(G):
                lo = g * 128
                m = min((g + 1) * 128, n) - lo
                if m <= 0:
                    continue
                c = c0 + g
                esrc0[:m, c] = ((sw[lo:lo + m] - k * NB) % NP).astype(np.int32)
                esrc1[:m, c] = sw[lo:lo + m].astype(np.int32)
                eslot[:m, c] = sl[lo:lo + m]
                eslotT[c * 128:c * 128 + m] = sl[lo:lo + m].astype(bf)
                evalid[:m, c * H:(c + 1) * H] = 1.0
                ets[c * 128:c * 128 + m] = tw[lo:lo + m]
        im = {
            "xT": np.roll(xpadT, -k * NB, axis=1).astype(bf),
            "esrc0": esrc0, "esrc1": esrc1,
            "eslot": eslot, "eslotT": eslotT, "evalid": evalid, "ets": ets,
        }
        im.update(consts)
        in_maps.append(im)

    meta = {"Tw": Tw, "colbase": colbase, "CT": CT, "CTpre": CTpre}
    return in_maps, meta


# ---------------------------------------------------------------- program

def _build_program(cfg, meta):
    NP, NB, W = cfg.NP, cfg.NB, cfg.W
    H, TD, IN = cfg.H, cfg.TD, cfg.IN
    HC0, HC1, D0, D1 = cfg.HC0, cfg.HC1, cfg.D0, cfg.D1
    Tw, colbase, CT, CTpre = meta["Tw"], meta["colbase"], meta["CT"], meta["CTpre"]
    Gmax = int(max(Tw))

    TB = BF if int(os.environ.get("KBF16", "1")) else FP
    nc = bacc.Bacc("TRN2", target_bir_lowering=False, debug=False,
                   num_devices=N_CORES)
    P = nc.declare_dram_parameter

    xT = P("xT", [IN, NP], TB, isOutput=False)
    esrc0 = P("esrc0", [128, CT], I32, isOutput=False)
    esrc1 = P("esrc1", [128, CT], I32, isOutput=False)
    eslot = P("eslot", [128, CT], FP, isOutput=False)
    eslotT = P("eslotT", [CT * 128], TB, isOutput=False)
    evalid = P("evalid", [128, CT * H], FP, isOutput=False)
    ets = P("ets", [CTpre * 128], FP, isOutput=False)
    W0ext_d = P("W0ext", [IN, HC0 + 8], TB, isOutput=False)
    W1A_d = P("W1extA", [128, HC1 + 8], TB, isOutput=False)
    W1B_d = P("W1extB", [128, HC1 + 8], TB, isOutput=False)
    M01_d = P("M01", [2 * TD, 2 * H], FP, isOutput=False)
    efsc_d = P("efsc", [128, 1], FP, isOutput=False)
    efbi_d = P("efbi", [128, 1], FP, isOutput=False)
    ones2_d = P("ones2", [2, 128], FP, isOutput=False)
    iota_d = P("iota", [128, 128], FP, isOutput=False)
    iotac_d = P("iotac", [128, 128], FP, isOutput=False)
    ident_d = P("ident", [128, 128], TB, isOutput=False)
    sl0_d = P("sl0", [128, H], FP, isOutput=False)
    sl1_d = P("sl1", [128, H], FP, isOutput=False)
    bias1_d = P("bias1", [128, HC1], FP, isOutput=False)
    c1ext_d = P("c1ext", [128, HC1 + 8], FP, isOutput=False)

    out_d = P("out", [NB, HC1], FP, isOutput=True)

    z0src = nc.dram_tensor("z0src", [NP, D0], TB)
    z0dst = nc.dram_tensor("z0dst", [NB, H], FP)
    z1loc = nc.dram_tensor("z1loc", [NB, D1], TB)
    z1dloc = nc.dram_tensor("z1dloc", [NB, H], FP)
    z1src_g = nc.dram_tensor("z1src_g", [NP, D1], TB, addr_space="Shared")

    AF = mybir.ActivationFunctionType
    OP = mybir.AluOpType
    TWO_PI = 2 * math.pi
    eslotT_v = eslotT[:].rearrange("(c p) -> c p", p=128)

    with ExitStack() as ctx:
        tc = ctx.enter_context(tile.TileContext(nc))
        cpool = ctx.enter_context(tc.tile_pool(name="consts", bufs=1))
        sepool = ctx.enter_context(tc.tile_pool(name="seall", bufs=1))
        sbuf = ctx.enter_context(tc.tile_pool(name="sbuf", bufs=3))
        gpool = ctx.enter_context(tc.tile_pool(name="gather", bufs=2))
        rpool = ctx.enter_context(tc.tile_pool(name="rhs", bufs=3))
        mpool = ctx.enter_context(tc.tile_pool(name="onehot", bufs=Gmax + 2))
        mtpool = ctx.enter_context(tc.tile_pool(name="onehot_t", bufs=3))
        fpool = ctx.enter_context(tc.tile_pool(name="flush", bufs=2))
        ppre = ctx.enter_context(tc.tile_pool(name="ppre", bufs=2, space="PSUM"))
        pse = ctx.enter_context(tc.tile_pool(name="pse", bufs=1, space="PSUM"))
        pseg = ctx.enter_context(tc.tile_pool(name="pseg", bufs=2, space="PSUM"))
        ptr = ctx.enter_context(tc.tile_pool(name="ptr", bufs=1, space="PSUM"))
        pz1 = ctx.enter_context(tc.tile_pool(name="pz1", bufs=1, space="PSUM"))

        def cload(dram, shape, tag, dtype=FP):
            t = cpool.tile(shape, dtype, tag=tag)
            nc.sync.dma_start(out=t[:], in_=dram[:])
            return t

        W0ext_s = cload(W0ext_d, [IN, HC0 + 8], "w0ext", TB)
        W1A_s = cload(W1A_d, [128, HC1 + 8], "w1a", TB)
        W1B_s = cload(W1B_d, [128, HC1 + 8], "w1b", TB)
        M01_s = cload(M01_d, [2 * TD, 2 * H], "m01")
        efsc_s = cload(efsc_d, [128, 1], "efsc")
        efbi_s = cload(efbi_d, [128, 1], "efbi")
        ones2_s = cload(ones2_d, [2, 128], "ones2")
        iota_s = cload(iota_d, [128, 128], "iota")
        iotac_s = cload(iotac_d, [128, 128], "iotac")
        ident_s = cload(ident_d, [128, 128], "ident", TB)
        sl0_s = cload(sl0_d, [128, H], "sl0")
        sl1_s = cload(sl1_d, [128, H], "sl1")
        bias1_s = cload(bias1_d, [128, HC1], "bias1")
        c1ext_s = cload(c1ext_d, [128, HC1 + 8], "c1ext")

        se_all = sepool.tile([128, CTpre * 2 * H], FP)

        PH = int(os.environ.get("KPHASES", "4"))
        KDEBUG = int(os.environ.get("KDEBUG", "0"))

        # ---- Phase Z0: rotated tables
        for t in range(NP // 128):
            xt = sbuf.tile([IN, 128], TB, tag="xt")
            nc.sync.dma_start(out=xt[:], in_=xT[:, t * 128:(t + 1) * 128])
            ps = pseg.tile([128, HC0 + 8], FP, tag="segp")
            nc.tensor.matmul(out=ps[:], lhsT=xt[:], rhs=W0ext_s[:],
                             start=True, stop=True)
            zt = sbuf.tile([128, D0], TB, tag="zt")
            nc.vector.tensor_copy(out=zt[:], in_=ps[:, :D0])
            nc.sync.dma_start(out=z0src[t * 128:(t + 1) * 128, :], in_=zt[:])
            if t < W:
                zt4 = sbuf.tile([128, H], FP, tag="zt4")
                nc.vector.tensor_copy(out=zt4[:], in_=ps[:, D0:D0 + H])
                nc.sync.dma_start(out=z0dst[t * 128:(t + 1) * 128, :],
                                  in_=zt4[:])

        # ---- Prephase: se_all
        ets_v = ets[:].rearrange("(a b) -> a b", b=512)
        for c in (range(0, CTpre, 8) if PH >= 2 else []):
            ts2 = sbuf.tile([2, 512], FP, tag="ts2")
            row = c * 128 // 512
            nc.sync.dma_start(out=ts2[:], in_=ets_v[row:row + 2, :])
            rp = ppre.tile([128, 512], FP, tag="prep")
            nc.tensor.matmul(out=rp[:], lhsT=ones2_s[:], rhs=ts2[:],
                             start=True, stop=True)
            t1 = sbuf.tile([128, 512], FP, tag="t1")
            nc.scalar.activation(out=t1[:], in_=rp[:], func=AF.Identity,
                                 bias=efbi_s[:], scale=efsc_s[:])
            k32 = sbuf.tile([128, 512], I32, tag="k32")
            nc.vector.tensor_copy(out=k32[:], in_=t1[:])
            kf = sbuf.tile([128, 512], FP, tag="kf")
            nc.vector.tensor_copy(out=kf[:], in_=k32[:])
            nc.vector.tensor_tensor(out=t1[:], in0=t1[:], in1=kf[:],
                                    op=OP.subtract)
            nc.scalar.activation(out=t1[:], in_=t1[:], func=AF.Sin,
                                 scale=TWO_PI)
            nc.scalar.activation(out=t1[:], in_=t1[:], func=AF.Abs)
            for b in range(8):
                half, coff = (b // 4) * 64, (b % 4) * 128
                sp = pse.tile([128, 2 * H], FP, tag="sep")
                nc.tensor.matmul(out=sp[:],
                                 lhsT=t1[half:half + 64, coff:coff + 128],
                                 rhs=M01_s[half:half + 64, :],
                                 start=True, stop=True)
                nc.vector.tensor_copy(
                    out=se_all[:, (c + b) * 2 * H:(c + b + 1) * 2 * H],
                    in_=sp[:])

        # ---- edge pass
        def edge_pass(layer):
            if layer == 0:
                table_s, dense_s, dense_d = z0src, z0src, z0dst
                esrc = esrc0
                DS, HC, seoff = D0, HC0, 0
            else:
                table_s, dense_s, dense_d = z1src_g, z1loc, z1dloc
                esrc = esrc1
                DS, HC, seoff = D1, HC1, H
            Cc = HC // H
            for w in range(W):
                G = int(Tw[w]); c0 = int(colbase[w])
                r0 = w * 128
                isrc = sbuf.tile([128, G], I32, tag="isrc")
                nc.sync.dma_start(out=isrc[:], in_=esrc[:, c0:c0 + G])
                slotf = sbuf.tile([128, G], FP, tag="slotf")
                nc.sync.dma_start(out=slotf[:], in_=eslot[:, c0:c0 + G])
                evw = sbuf.tile([128, G * H], FP, tag="evw")
                nc.sync.dma_start(out=evw[:],
                                  in_=evalid[:, c0 * H:(c0 + G) * H])
                # dense own-window rows (self loops + ad_w)
                zw = fpool.tile([128, DS], TB, tag=f"zw{layer}")
                nc.sync.dma_start(out=zw[:], in_=dense_s[r0:r0 + 128, :])
                adw = fpool.tile([128, H], FP, tag="adw")
                nc.sync.dma_start(out=adw[:], in_=dense_d[r0:r0 + 128, :])
                adw_bf = fpool.tile([128, H], TB, tag="adwbf")
                nc.vector.tensor_copy(out=adw_bf[:], in_=adw[:])

                zg = gpool.tile([128, G * DS], TB, tag=f"zg{layer}")
                Ms = []
                adps = pse.tile([128, G * H], FP, tag="adps")
                for g in range(G):
                    nc.gpsimd.indirect_dma_start(
                        out=zg[:, g * DS:(g + 1) * DS], out_offset=None,
                        in_=table_s[:],
                        in_offset=bass.IndirectOffsetOnAxis(
                            ap=isrc[:, g:g + 1], axis=0))
                    # one-hot M[e, slot] (bf16)
                    M = mpool.tile([128, 128], TB, tag="M")
                    nc.vector.tensor_tensor(
                        out=M[:],
                        in0=slotf[:, g:g + 1].to_broadcast([128, 128]),
                        in1=iota_s[:], op=OP.is_equal)
                    Ms.append(M)
                    # transposed one-hot Mt[s, e] from a broadcast slot row
                    srep = mtpool.tile([128, 128], TB, tag="srep")
                    nc.scalar.dma_start(
                        out=srep[:],
                        in_=eslotT_v[c0 + g:c0 + g + 1, :].to_broadcast(
                            [128, 128]))
                    Mt = mtpool.tile([128, 128], TB, tag="Mt")
                    nc.vector.tensor_tensor(out=Mt[:], in0=srep[:],
                                            in1=iotac_s[:], op=OP.is_equal)
                    # ad[dst] per edge -> adps[:, g*H:(g+1)*H]
                    nc.tensor.matmul(out=adps[:, g * H:(g + 1) * H],
                                     lhsT=Mt[:], rhs=adw_bf[:],
                                     start=True, stop=True)

                zg_v = zg[:].rearrange("p (g d) -> p g d", d=DS)
                se_v = se_all[:, c0 * 2 * H:(c0 + G) * 2 * H].rearrange(
                    "p (g e) -> p g e", e=2 * H)

                # alpha = se + as[src] + ad[dst]; lrelu; exp; mask padding
                alpha = sbuf.tile([128, G * H], FP, tag="alpha")
                al_v = alpha[:].rearrange("p (g h) -> p g h", h=H)
                nc.vector.tensor_tensor(
                    out=al_v, in0=se_v[:, :, seoff:seoff + H],
                    in1=zg_v[:, :, HC:HC + H], op=OP.add)
                nc.vector.tensor_tensor(
                    out=alpha[:], in0=alpha[:], in1=adps[:], op=OP.add)
                scaled = sbuf.tile([128, G * H], FP, tag="scaled")
                nc.vector.tensor_scalar(out=scaled[:], in0=alpha[:],
                                        scalar1=SLOPE, scalar2=None,
                                        op0=OP.mult)
                nc.vector.tensor_tensor(out=alpha[:], in0=alpha[:],
                                        in1=scaled[:], op=OP.max)
                expw = sbuf.tile([128, G * H], FP, tag="expw")
                nc.scalar.activation(out=expw[:], in_=alpha[:], func=AF.Exp)
                nc.vector.tensor_tensor(out=expw[:], in0=expw[:],
                                        in1=evw[:], op=OP.mult)
                ex_v = expw[:].rearrange("p (g h) -> p g h", h=H)

                ps = pseg.tile([128, HC + H], FP, tag="segp")
                for g in range(G):
                    rhs = rpool.tile([128, HC + H], TB, tag=f"rhs{layer}")
                    nc.vector.tensor_tensor(
                        out=rhs[:, :HC].rearrange("p (h c) -> p h c", c=Cc),
                        in0=zg_v[:, g, :HC].rearrange("p (h c) -> p h c",
                                                      c=Cc),
                        in1=ex_v[:, g, :].rearrange(
                            "p (h o) -> p h o", o=1).to_broadcast(
                                [128, H, Cc]),
                        op=OP.mult)
                    nc.scalar.copy(out=rhs[:, HC:HC + H],
                                   in_=expw[:, g * H:(g + 1) * H])
                    nc.tensor.matmul(out=ps[:], lhsT=Ms[g][:], rhs=rhs[:],
                                     start=(g == 0), stop=(g == G - 1))

                # ---- flush
                sl_s = sl0_s if layer == 0 else sl1_s
                asel = fpool.tile([128, H], FP, tag="asel")
                nc.vector.tensor_tensor(out=asel[:], in0=zw[:, HC:HC + H],
                                        in1=adw[:], op=OP.add)
                nc.vector.tensor_tensor(out=asel[:], in0=asel[:],
                                        in1=sl_s[:], op=OP.add)
                ssc = fpool.tile([128, H], FP, tag="ssc")
                nc.vector.tensor_scalar(out=ssc[:], in0=asel[:], scalar1=SLOPE,
                                        scalar2=None, op0=OP.mult)
                nc.vector.tensor_tensor(out=asel[:], in0=asel[:], in1=ssc[:],
                                        op=OP.max)
                nc.scalar.activation(out=asel[:], in_=asel[:], func=AF.Exp)
                den = fpool.tile([128, H], FP, tag="den")
                nc.vector.tensor_tensor(out=den[:], in0=ps[:, HC:HC + H],
                                        in1=asel[:], op=OP.add)
                rec = fpool.tile([128, H], FP, tag="rec")
                nc.vector.reciprocal(out=rec[:], in_=den[:])
                o0 = fpool.tile([128, HC], FP, tag=f"o0{layer}")
                for h in range(H):
                    hs = slice(h * Cc, (h + 1) * Cc)
                    nc.vector.tensor_scalar(
                        out=o0[:, hs], in0=zw[:, hs],
                        scalar1=asel[:, h:h + 1], scalar2=None, op0=OP.mult)
                    nc.vector.tensor_tensor(out=o0[:, hs], in0=o0[:, hs],
                                            in1=ps[:, hs], op=OP.add)
                    nc.vector.tensor_scalar(
                        out=o0[:, hs], in0=o0[:, hs],
                        scalar1=rec[:, h:h + 1], scalar2=None, op0=OP.mult)

                if layer == 0:
                    o0b = fpool.tile([128, HC], TB, tag="o0b")
                    nc.vector.tensor_copy(out=o0b[:], in_=o0[:])
                    z1p = pz1.tile([128, HC1 + 8], FP, tag="z1p")
                    for half, Wh in ((0, W1A_s), (1, W1B_s)):
                        tp = ptr.tile([128, 128], TB, tag="tp")
                        nc.tensor.transpose(
                            out=tp[:],
                            in_=o0b[:, half * 128:(half + 1) * 128],
                            identity=ident_s[:])
                        oT = fpool.tile([128, 128], TB, tag="oT")
                        nc.vector.tensor_copy(out=oT[:], in_=tp[:])
                        nc.tensor.matmul(out=z1p[:], lhsT=oT[:], rhs=Wh[:],
                                         start=(half == 0), stop=(half == 1))
                    z1e = fpool.tile([128, D1], TB, tag="z1e")
                    nc.vector.tensor_tensor(out=z1e[:], in0=z1p[:, :D1],
                                            in1=c1ext_s[:, :D1], op=OP.add)
                    nc.sync.dma_start(out=z1loc[r0:r0 + 128, :], in_=z1e[:])
                    z1e4 = fpool.tile([128, H], FP, tag="z1e4")
                    nc.vector.tensor_tensor(out=z1e4[:],
                                            in0=z1p[:, D1:D1 + H],
                                            in1=c1ext_s[:, D1:D1 + H],
                                            op=OP.add)
                    nc.sync.dma_start(out=z1dloc[r0:r0 + 128, :], in_=z1e4[:])
                else:
                    nc.vector.tensor_tensor(out=o0[:], in0=o0[:],
                                            in1=bias1_s[:], op=OP.add)
                    nc.sync.dma_start(out=out_d[r0:r0 + 128, :], in_=o0[:])

        if PH >= 3:
            edge_pass(0)

        if PH >= 4:
            nc.gpsimd.collective_compute(
                "AllGather", mybir.AluOpType.bypass,
                replica_groups=[list(range(N_CORES))],
                ins=[z1loc[:]], outs=[z1src_g[:]])

            edge_pass(1)

    nc.compile()
    return nc


# ---------------------------------------------------------------- entry

def kernel(**inputs):
    cfg = CFG(N=inputs["x"].shape[0], E=inputs["edge_index"].shape[1],
              IN=inputs["x"].shape[1], TD=inputs["time_w"].shape[0],
              H=np.asarray(inputs["att_src0"]).shape[0],
              C0=np.asarray(inputs["att_src0"]).shape[1],
              C1=np.asarray(inputs["att_src1"]).shape[1])
    in_maps, meta = _host_prep(cfg, **{k: np.asarray(v) for k, v in inputs.items()})
    nc = _build_program(cfg, meta)
    res = run_bass_kernel_spmd(nc, in_maps, list(range(N_CORES)))
    blocks = [res.results[k]["out"] for k in range(N_CORES)]
    return np.concatenate(blocks, axis=0)[:cfg.N].astype(np.float32)


# revision 19
# speedup vs baseline: 1.4310x; 1.2089x over previous
"""Trainium2 Bass kernel for nn_GAT_Simple (2-layer GATConv with time-encoded
edge attrs, 50k nodes / 800k edges, 8 NeuronCores).

Sharding: nodes partitioned into 8 blocks; core k owns destination block k.
Per-core tables are node-ROTATED so each core's own block sits at rows [0,NB)
of its local tables, keeping the SPMD program core-independent.

Phases per core:
  Z0       - full (rotated) projected tables z0src = bf16 [x@W0 | alpha_src
             terms], z0dst = fp32 alpha_dst terms for the local block only.
  prephase - per-edge time-encoder attention scalars se = |cos(ts*w+b)| @ M
             for BOTH layers into a resident SBUF buffer (keeps the Sin ACT
             table set out of the edge loop, which uses the Exp set).
  L0 edges - per 128-dst-node window: per-tile 128-row indirect gathers of
             bf16 source rows; alpha -> leaky-relu -> exp; ad[dst] expanded
             from a dense per-window vector via a transposed one-hot matmul
             accumulated in PSUM; weighted segment-sum + softmax denominator
             via a one-hot matmul per tile accumulated in PSUM (normalize
             after aggregation => single pass over edges). Window flush adds
             self-loops densely, normalizes, and fuses the layer-1 projection.
  AllGather of per-block bf16 z1 tables across the 8 cores.
  L1 edges - same structure on the gathered z1 tables; adds bias1; writes the
             per-core fp32 output block.
"""

import math
import os
from contextlib import ExitStack

import numpy as np

try:
    import ml_dtypes  # noqa: F401  (registers bfloat16 with numpy)
    _BF16 = np.dtype("bfloat16")
except ImportError:
    _BF16 = None

import concourse.bass as bass
import concourse.bacc as bacc
import concourse.tile as tile
from concourse import mybir
from concourse.bass_utils import run_bass_kernel_spmd

FP = mybir.dt.float32
BF = mybir.dt.bfloat16
I32 = mybir.dt.int32

N_CORES = 8
SLOPE = 0.2


class CFG:
    def __init__(self, N=50000, E=800000, IN=128, TD=64, H=4, C0=64, C1=16):
        self.N, self.E, self.IN, self.TD = N, E, IN, TD
        self.H, self.C0, self.C1 = H, C0, C1
        self.HC0, self.HC1 = H * C0, H * C1              # 256, 64
        self.D0 = self.HC0 + 4                            # z0 | as0  -> 260
        self.D1 = self.HC1 + 4                            # z1 | as1  -> 68
        self.NP = ((N + N_CORES * 128 - 1) // (N_CORES * 128)) * (N_CORES * 128)
        self.NB = self.NP // N_CORES
        self.W = self.NB // 128


# ---------------------------------------------------------------- host prep

def _host_prep(cfg, x, edge_index, timestamps, time_w, time_b,
               W0, att_src0, att_dst0, lin_edge0, att_edge0, bias0,
               W1, att_src1, att_dst1, lin_edge1, att_edge1, bias1):
    N, NP, NB, W = cfg.N, cfg.NP, cfg.NB, cfg.W
    H, C0, C1, TD, IN = cfg.H, cfg.C0, cfg.C1, cfg.TD, cfg.IN
    HC0, HC1 = cfg.HC0, cfg.HC1

    f32 = np.float32
    bf = _BF16 if int(os.environ.get("KBF16", "1")) else f32
    x = np.asarray(x, f32)
    src = np.asarray(edge_index[0], np.int64)
    dst = np.asarray(edge_index[1], np.int64)
    ts = np.asarray(timestamps, f32)

    def blockdiag(att, C):  # [H, C] -> [H*C, H]
        out = np.zeros((H * C, H), f32)
        for h in range(H):
            out[h * C:(h + 1) * C, h] = att[h]
        return out

    W0 = np.asarray(W0, f32); W1 = np.asarray(W1, f32)
    W0ext = np.concatenate(
        [W0, W0 @ blockdiag(np.asarray(att_src0, f32), C0),
         W0 @ blockdiag(np.asarray(att_dst0, f32), C0)], axis=1)  # [IN, HC0+8]
    W1ext = np.concatenate(
        [W1, W1 @ blockdiag(np.asarray(att_src1, f32), C1),
         W1 @ blockdiag(np.asarray(att_dst1, f32), C1)], axis=1)  # [HC0, HC1+8]
    c1ext = np.asarray(bias0, f32) @ W1ext                         # [HC1+8]

    M0c = np.einsum("thc,hc->th",
                    np.asarray(lin_edge0, f32).reshape(TD, H, C0),
                    np.asarray(att_edge0, f32))
    M1c = np.einsum("thc,hc->th",
                    np.asarray(lin_edge1, f32).reshape(TD, H, C1),
                    np.asarray(att_edge1, f32))
    M01 = np.concatenate([M0c, M1c], axis=1).astype(f32)           # [TD, 2H]
    M01x2 = np.concatenate([M01, M01], axis=0)                     # [2TD, 2H]
    sl0 = M0c.sum(axis=0).astype(f32)
    sl1 = M1c.sum(axis=0).astype(f32)

    # |cos(ts*w+b)| = |sin(2*pi*r)|, r = t1 - round(t1),
    # t1 = ts*(w/2pi) + (b+pi/2)/(2pi)
    w2pi = (np.asarray(time_w, f32) / (2 * np.pi)).astype(f32)
    b2pi = ((np.asarray(time_b, f32) + np.pi / 2) / (2 * np.pi)).astype(f32)
    efsc = np.tile(w2pi, 2).reshape(128, 1)
    efbi = np.tile(b2pi, 2).reshape(128, 1)

    ones2 = np.zeros((2, 128), f32)
    ones2[0, :64] = 1.0
    ones2[1, 64:] = 1.0

    iota_row = np.broadcast_to(np.arange(128, dtype=f32), (128, 128)).copy()
    iota_col = np.broadcast_to(np.arange(128, dtype=f32)[:, None],
                               (128, 128)).copy()
    identity_bf = np.eye(128).astype(bf)

    xpadT = np.zeros((IN, NP), f32)
    xpadT[:, :N] = x.T

    blk = dst // NB
    per_core = []
    for k in range(N_CORES):
        m = blk == k
        s_k, d_k, t_k = src[m], dst[m], ts[m]
        order = np.argsort(d_k, kind="stable")
        per_core.append((s_k[order], d_k[order], t_k[order]))

    cnt = np.zeros((N_CORES, W), np.int64)
    for k in range(N_CORES):
        d_k = per_core[k][1] - k * NB
        cnt[k] = np.bincount(d_k // 128, minlength=W)
    Tw = np.maximum(1, (cnt.max(axis=0) + 127) // 128).astype(np.int64)
    CT = int(Tw.sum())
    CTpre = ((CT + 7) // 8) * 8
    colbase = np.zeros(W + 1, np.int64)
    colbase[1:] = np.cumsum(Tw)

    consts = {
        "W0ext": W0ext,
        "W1extA": W1ext[:128].astype(bf), "W1extB": W1ext[128:].astype(bf),
        "M01": M01x2, "efsc": efsc, "efbi": efbi, "ones2": ones2,
        "iota": iota_row, "iotac": iota_col, "ident": identity_bf,
        "sl0": np.broadcast_to(sl0, (128, H)).copy(),
        "sl1": np.broadcast_to(sl1, (128, H)).copy(),
        "bias1": np.broadcast_to(np.asarray(bias1, f32), (128, HC1)).copy(),
        "c1ext": np.broadcast_to(c1ext, (128, HC1 + 8)).copy(),
    }

    in_maps = []
    for k in range(N_CORES):
        s_k, d_k, t_k = per_core[k]
        esrc0 = np.zeros((128, CT), np.int32)
        esrc1 = np.zeros((128, CT), np.int32)
        eslot = np.zeros((128, CT), f32)
        eslotT = np.zeros(CT * 128, bf)
        evalid = np.zeros((128, CT * H), f32)
        ets = np.zeros(CTpre * 128, f32)

        pos = 0
        for w in range(W):
            n = int(cnt[k, w])
            sw = s_k[pos:pos + n]; dw = d_k[pos:pos + n]; tw = t_k[pos:pos + n]
            pos += n
            G = int(Tw[w]); c0 = int(colbase[w])
            sl = (dw - k * NB - w * 128).astype(f32)
            for g in range(G):
                lo = g * 128
                m = min((g + 1) * 128, n) - lo
                if m <= 0:
                    continue
                c = c0 + g
                esrc0[:m, c] = ((sw[lo:lo + m] - k * NB) % NP).astype(np.int32)
                esrc1[:m, c] = sw[lo:lo + m].astype(np.int32)
                eslot[:m, c] = sl[lo:lo + m]
                eslotT[c * 128:c * 128 + m] = sl[lo:lo + m].astype(bf)
                evalid[:m, c * H:(c + 1) * H] = 1.0
                ets[c * 128:c * 128 + m] = tw[lo:lo + m]
        im = {
            "xT": np.roll(xpadT, -k * NB, axis=1),
            "esrc0": esrc0, "esrc1": esrc1,
            "eslot": eslot, "eslotT": eslotT, "evalid": evalid, "ets": ets,
        }
        im.update(consts)
        in_maps.append(im)

    meta = {"Tw": Tw, "colbase": colbase, "CT": CT, "CTpre": CTpre}
    return in_maps, meta


# ---------------------------------------------------------------- program

def _build_program(cfg, meta):
    NP, NB, W = cfg.NP, cfg.NB, cfg.W
    H, TD, IN = cfg.H, cfg.TD, cfg.IN
    HC0, HC1, D0, D1 = cfg.HC0, cfg.HC1, cfg.D0, cfg.D1
    Tw, colbase, CT, CTpre = meta["Tw"], meta["colbase"], meta["CT"], meta["CTpre"]
    Gmax = int(max(Tw))

    TB = BF if int(os.environ.get("KBF16", "1")) else FP
    nc = bacc.Bacc("TRN2", target_bir_lowering=False, debug=False,
                   num_devices=N_CORES)
    P = nc.declare_dram_parameter

    xT = P("xT", [IN, NP], FP, isOutput=False)
    esrc0 = P("esrc0", [128, CT], I32, isOutput=False)
    esrc1 = P("esrc1", [128, CT], I32, isOutput=False)
    eslot = P("eslot", [128, CT], FP, isOutput=False)
    eslotT = P("eslotT", [CT * 128], TB, isOutput=False)
    evalid = P("evalid", [128, CT * H], FP, isOutput=False)
    ets = P("ets", [CTpre * 128], FP, isOutput=False)
    W0ext_d = P("W0ext", [IN, HC0 + 8], FP, isOutput=False)
    W1A_d = P("W1extA", [128, HC1 + 8], TB, isOutput=False)
    W1B_d = P("W1extB", [128, HC1 + 8], TB, isOutput=False)
    M01_d = P("M01", [2 * TD, 2 * H], FP, isOutput=False)
    efsc_d = P("efsc", [128, 1], FP, isOutput=False)
    efbi_d = P("efbi", [128, 1], FP, isOutput=False)
    ones2_d = P("ones2", [2, 128], FP, isOutput=False)
    iota_d = P("iota", [128, 128], FP, isOutput=False)
    iotac_d = P("iotac", [128, 128], FP, isOutput=False)
    ident_d = P("ident", [128, 128], TB, isOutput=False)
    sl0_d = P("sl0", [128, H], FP, isOutput=False)
    sl1_d = P("sl1", [128, H], FP, isOutput=False)
    bias1_d = P("bias1", [128, HC1], FP, isOutput=False)
    c1ext_d = P("c1ext", [128, HC1 + 8], FP, isOutput=False)

    out_d = P("out", [NB, HC1], FP, isOutput=True)

    z0src = nc.dram_tensor("z0src", [NP, D0], TB)
    z0dst = nc.dram_tensor("z0dst", [NB, H], FP)
    z1loc = nc.dram_tensor("z1loc", [NB, D1], TB)
    z1dloc = nc.dram_tensor("z1dloc", [NB, H], FP)
    z1src_g = nc.dram_tensor("z1src_g", [NP, D1], TB, addr_space="Shared")

    AF = mybir.ActivationFunctionType
    OP = mybir.AluOpType
    TWO_PI = 2 * math.pi
    eslotT_v = eslotT[:].rearrange("(c p) -> c p", p=128)

    with ExitStack() as ctx:
        tc = ctx.enter_context(tile.TileContext(nc))
        cpool = ctx.enter_context(tc.tile_pool(name="consts", bufs=1))
        sepool = ctx.enter_context(tc.tile_pool(name="seall", bufs=1))
        sbuf = ctx.enter_context(tc.tile_pool(name="sbuf", bufs=3))
        gpool = ctx.enter_context(tc.tile_pool(name="gather", bufs=2))
        rpool = ctx.enter_context(tc.tile_pool(name="rhs", bufs=3))
        mpool = ctx.enter_context(tc.tile_pool(name="onehot", bufs=2))
        mtpool = ctx.enter_context(tc.tile_pool(name="onehot_t", bufs=2))
        fpool = ctx.enter_context(tc.tile_pool(name="flush", bufs=2))
        ppre = ctx.enter_context(tc.tile_pool(name="ppre", bufs=2, space="PSUM"))
        pse = ctx.enter_context(tc.tile_pool(name="pse", bufs=1, space="PSUM"))
        pseg = ctx.enter_context(tc.tile_pool(name="pseg", bufs=2, space="PSUM"))
        ptr = ctx.enter_context(tc.tile_pool(name="ptr", bufs=1, space="PSUM"))
        pz1 = ctx.enter_context(tc.tile_pool(name="pz1", bufs=1, space="PSUM"))

        def cload(dram, shape, tag, dtype=FP):
            t = cpool.tile(shape, dtype, tag=tag)
            nc.sync.dma_start(out=t[:], in_=dram[:])
            return t

        W0ext_s = cload(W0ext_d, [IN, HC0 + 8], "w0ext")
        W1A_s = cload(W1A_d, [128, HC1 + 8], "w1a", TB)
        W1B_s = cload(W1B_d, [128, HC1 + 8], "w1b", TB)
        M01_s = cload(M01_d, [2 * TD, 2 * H], "m01")
        efsc_s = cload(efsc_d, [128, 1], "efsc")
        efbi_s = cload(efbi_d, [128, 1], "efbi")
        ones2_s = cload(ones2_d, [2, 128], "ones2")
        iota_s = cload(iota_d, [128, 128], "iota")
        iotac_s = cload(iotac_d, [128, 128], "iotac")
        ident_s = cload(ident_d, [128, 128], "ident", TB)
        sl0_s = cload(sl0_d, [128, H], "sl0")
        sl1_s = cload(sl1_d, [128, H], "sl1")
        bias1_s = cload(bias1_d, [128, HC1], "bias1")
        c1ext_s = cload(c1ext_d, [128, HC1 + 8], "c1ext")

        se_all = sepool.tile([128, CTpre * 2 * H], FP)

        PH = int(os.environ.get("KPHASES", "4"))
        KDEBUG = int(os.environ.get("KDEBUG", "0"))

        # ---- Phase Z0: rotated tables
        for t in range(NP // 128):
            xt = sbuf.tile([IN, 128], FP, tag="xt")
            nc.sync.dma_start(out=xt[:], in_=xT[:, t * 128:(t + 1) * 128])
            ps = pseg.tile([128, HC0 + 8], FP, tag="segp")
            nc.tensor.matmul(out=ps[:], lhsT=xt[:], rhs=W0ext_s[:],
                             start=True, stop=True)
            zt = sbuf.tile([128, D0], TB, tag="zt")
            nc.vector.tensor_copy(out=zt[:], in_=ps[:, :D0])
            nc.sync.dma_start(out=z0src[t * 128:(t + 1) * 128, :], in_=zt[:])
            if t < W:
                zt4 = sbuf.tile([128, H], FP, tag="zt4")
                nc.vector.tensor_copy(out=zt4[:], in_=ps[:, D0:D0 + H])
                nc.sync.dma_start(out=z0dst[t * 128:(t + 1) * 128, :],
                                  in_=zt4[:])

        # ---- Prephase: se_all
        ets_v = ets[:].rearrange("(a b) -> a b", b=512)
        for c in (range(0, CTpre, 8) if PH >= 2 else []):
            ts2 = sbuf.tile([2, 512], FP, tag="ts2")
            row = c * 128 // 512
            nc.sync.dma_start(out=ts2[:], in_=ets_v[row:row + 2, :])
            rp = ppre.tile([128, 512], FP, tag="prep")
            nc.tensor.matmul(out=rp[:], lhsT=ones2_s[:], rhs=ts2[:],
                             start=True, stop=True)
            t1 = sbuf.tile([128, 512], FP, tag="t1")
            nc.scalar.activation(out=t1[:], in_=rp[:], func=AF.Identity,
                                 bias=efbi_s[:], scale=efsc_s[:])
            k32 = sbuf.tile([128, 512], I32, tag="k32")
            nc.vector.tensor_copy(out=k32[:], in_=t1[:])
            kf = sbuf.tile([128, 512], FP, tag="kf")
            nc.vector.tensor_copy(out=kf[:], in_=k32[:])
            nc.vector.tensor_tensor(out=t1[:], in0=t1[:], in1=kf[:],
                                    op=OP.subtract)
            nc.scalar.activation(out=t1[:], in_=t1[:], func=AF.Sin,
                                 scale=TWO_PI)
            nc.scalar.activation(out=t1[:], in_=t1[:], func=AF.Abs)
            for b in range(8):
                half, coff = (b // 4) * 64, (b % 4) * 128
                sp = pse.tile([128, 2 * H], FP, tag="sep")
                nc.tensor.matmul(out=sp[:],
                                 lhsT=t1[half:half + 64, coff:coff + 128],
                                 rhs=M01_s[half:half + 64, :],
                                 start=True, stop=True)
                nc.vector.tensor_copy(
                    out=se_all[:, (c + b) * 2 * H:(c + b + 1) * 2 * H],
                    in_=sp[:])

        # ---- edge pass
        def edge_pass(layer):
            if layer == 0:
                table_s, dense_s, dense_d = z0src, z0src, z0dst
                esrc = esrc0
                DS, HC, seoff = D0, HC0, 0
            else:
                table_s, dense_s, dense_d = z1src_g, z1loc, z1dloc
                esrc = esrc1
                DS, HC, seoff = D1, HC1, H
            Cc = HC // H
            for w in range(W):
                G = int(Tw[w]); c0 = int(colbase[w])
                r0 = w * 128
                isrc = sbuf.tile([128, G], I32, tag="isrc")
                nc.sync.dma_start(out=isrc[:], in_=esrc[:, c0:c0 + G])
                slotf = sbuf.tile([128, G], FP, tag="slotf")
                nc.sync.dma_start(out=slotf[:], in_=eslot[:, c0:c0 + G])
                evw = sbuf.tile([128, G * H], FP, tag="evw")
                nc.sync.dma_start(out=evw[:],
                                  in_=evalid[:, c0 * H:(c0 + G) * H])
                # dense own-window rows (self loops + ad_w)
                zw = fpool.tile([128, DS], TB, tag=f"zw{layer}")
                nc.sync.dma_start(out=zw[:], in_=dense_s[r0:r0 + 128, :])
                adw = fpool.tile([128, H], FP, tag="adw")
                nc.sync.dma_start(out=adw[:], in_=dense_d[r0:r0 + 128, :])
                adw_bf = fpool.tile([128, H], TB, tag="adwbf")
                nc.vector.tensor_copy(out=adw_bf[:], in_=adw[:])

                zg = gpool.tile([128, G * DS], TB, tag=f"zg{layer}")
                for g in range(G):
                    nc.gpsimd.indirect_dma_start(
                        out=zg[:, g * DS:(g + 1) * DS], out_offset=None,
                        in_=table_s[:],
                        in_offset=bass.IndirectOffsetOnAxis(
                            ap=isrc[:, g:g + 1], axis=0))
                # one-hot M_all[e, (g, slot)] in one DVE op
                M_all = mpool.tile([128, G * 128], TB, tag="M")
                nc.vector.tensor_tensor(
                    out=M_all[:].rearrange("p (g s) -> p g s", s=128),
                    in0=slotf[:].rearrange("p (g o) -> p g o",
                                           o=1).to_broadcast([128, G, 128]),
                    in1=iota_s[:].rearrange("p (o s) -> p o s",
                                            o=1).to_broadcast([128, G, 128]),
                    op=OP.is_equal)
                # transposed one-hots: one broadcast DMA + one is_eq
                srep = mtpool.tile([128, G * 128], TB, tag="srep")
                nc.scalar.dma_start(
                    out=srep[:].rearrange("p (g e) -> p g e", e=128),
                    in_=eslotT_v[c0:c0 + G, :].rearrange(
                        "g (o e) -> o g e", o=1).to_broadcast([128, G, 128]))
                Mt_all = mtpool.tile([128, G * 128], TB, tag="Mt")
                nc.vector.tensor_tensor(
                    out=Mt_all[:].rearrange("p (g e) -> p g e", e=128),
                    in0=srep[:].rearrange("p (g e) -> p g e", e=128),
                    in1=iotac_s[:].rearrange("p (o e) -> p o e",
                                             o=1).to_broadcast([128, G, 128]),
                    op=OP.is_equal)
                adps = pse.tile([128, G * H], FP, tag="adps")
                for g in range(G):
                    nc.tensor.matmul(out=adps[:, g * H:(g + 1) * H],
                                     lhsT=Mt_all[:, g * 128:(g + 1) * 128],
                                     rhs=adw_bf[:], start=True, stop=True)

                zg_v = zg[:].rearrange("p (g d) -> p g d", d=DS)
                se_v = se_all[:, c0 * 2 * H:(c0 + G) * 2 * H].rearrange(
                    "p (g e) -> p g e", e=2 * H)

                # alpha = se + as[src] + ad[dst]; lrelu; exp; mask padding
                alpha = sbuf.tile([128, G * H], FP, tag="alpha")
                al_v = alpha[:].rearrange("p (g h) -> p g h", h=H)
                nc.vector.tensor_tensor(
                    out=al_v, in0=se_v[:, :, seoff:seoff + H],
                    in1=zg_v[:, :, HC:HC + H], op=OP.add)
                nc.vector.tensor_tensor(
                    out=alpha[:], in0=alpha[:], in1=adps[:], op=OP.add)
                scaled = sbuf.tile([128, G * H], FP, tag="scaled")
                nc.vector.tensor_scalar(out=scaled[:], in0=alpha[:],
                                        scalar1=SLOPE, scalar2=None,
                                        op0=OP.mult)
                nc.vector.tensor_tensor(out=alpha[:], in0=alpha[:],
                                        in1=scaled[:], op=OP.max)
                expw = sbuf.tile([128, G * H], FP, tag="expw")
                nc.scalar.activation(out=expw[:], in_=alpha[:], func=AF.Exp)
                nc.vector.tensor_tensor(out=expw[:], in0=expw[:],
                                        in1=evw[:], op=OP.mult)
                ex_v = expw[:].rearrange("p (g h) -> p g h", h=H)

                ps = pseg.tile([128, HC + H], FP, tag="segp")
                for g in range(G):
                    rhs = rpool.tile([128, HC + H], TB, tag=f"rhs{layer}")
                    nc.vector.tensor_tensor(
                        out=rhs[:, :HC].rearrange("p (h c) -> p h c", c=Cc),
                        in0=zg_v[:, g, :HC].rearrange("p (h c) -> p h c",
                                                      c=Cc),
                        in1=ex_v[:, g, :].rearrange(
                            "p (h o) -> p h o", o=1).to_broadcast(
                                [128, H, Cc]),
                        op=OP.mult)
                    nc.scalar.copy(out=rhs[:, HC:HC + H],
                                   in_=expw[:, g * H:(g + 1) * H])
                    nc.tensor.matmul(out=ps[:],
                                     lhsT=M_all[:, g * 128:(g + 1) * 128],
                                     rhs=rhs[:],
                                     start=(g == 0), stop=(g == G - 1))

                # ---- flush
                sl_s = sl0_s if layer == 0 else sl1_s
                asel = fpool.tile([128, H], FP, tag="asel")
                nc.vector.tensor_tensor(out=asel[:], in0=zw[:, HC:HC + H],
                                        in1=adw[:], op=OP.add)
                nc.vector.tensor_tensor(out=asel[:], in0=asel[:],
                                        in1=sl_s[:], op=OP.add)
                ssc = fpool.tile([128, H], FP, tag="ssc")
                nc.vector.tensor_scalar(out=ssc[:], in0=asel[:], scalar1=SLOPE,
                                        scalar2=None, op0=OP.mult)
                nc.vector.tensor_tensor(out=asel[:], in0=asel[:], in1=ssc[:],
                                        op=OP.max)
                nc.scalar.activation(out=asel[:], in_=asel[:], func=AF.Exp)
                den = fpool.tile([128, H], FP, tag="den")
                nc.vector.tensor_tensor(out=den[:], in0=ps[:, HC:HC + H],
                                        in1=asel[:], op=OP.add)
                rec = fpool.tile([128, H], FP, tag="rec")
                nc.vector.reciprocal(out=rec[:], in_=den[:])
                o0 = fpool.tile([128, HC], FP, tag=f"o0{layer}")
                for h in range(H):
                    hs = slice(h * Cc, (h + 1) * Cc)
                    nc.vector.tensor_scalar(
                        out=o0[:, hs], in0=zw[:, hs],
                        scalar1=asel[:, h:h + 1], scalar2=None, op0=OP.mult)
                    nc.vector.tensor_tensor(out=o0[:, hs], in0=o0[:, hs],
                                            in1=ps[:, hs], op=OP.add)
                    nc.vector.tensor_scalar(
                        out=o0[:, hs], in0=o0[:, hs],
                        scalar1=rec[:, h:h + 1], scalar2=None, op0=OP.mult)

                if layer == 0:
                    o0b = fpool.tile([128, HC], TB, tag="o0b")
                    nc.vector.tensor_copy(out=o0b[:], in_=o0[:])
                    z1p = pz1.tile([128, HC1 + 8], FP, tag="z1p")
                    for half, Wh in ((0, W1A_s), (1, W1B_s)):
                        tp = ptr.tile([128, 128], TB, tag="tp")
                        nc.tensor.transpose(
                            out=tp[:],
                            in_=o0b[:, half * 128:(half + 1) * 128],
                            identity=ident_s[:])
                        oT = fpool.tile([128, 128], TB, tag="oT")
                        nc.vector.tensor_copy(out=oT[:], in_=tp[:])
                        nc.tensor.matmul(out=z1p[:], lhsT=oT[:], rhs=Wh[:],
                                         start=(half == 0), stop=(half == 1))
                    z1e = fpool.tile([128, D1], TB, tag="z1e")
                    nc.vector.tensor_tensor(out=z1e[:], in0=z1p[:, :D1],
                                            in1=c1ext_s[:, :D1], op=OP.add)
                    nc.sync.dma_start(out=z1loc[r0:r0 + 128, :], in_=z1e[:])
                    z1e4 = fpool.tile([128, H], FP, tag="z1e4")
                    nc.vector.tensor_tensor(out=z1e4[:],
                                            in0=z1p[:, D1:D1 + H],
                                            in1=c1ext_s[:, D1:D1 + H],
                                            op=OP.add)
                    nc.sync.dma_start(out=z1dloc[r0:r0 + 128, :], in_=z1e4[:])
                else:
                    nc.vector.tensor_tensor(out=o0[:], in0=o0[:],
                                            in1=bias1_s[:], op=OP.add)
                    nc.sync.dma_start(out=out_d[r0:r0 + 128, :], in_=o0[:])

        if PH >= 3:
            edge_pass(0)

        if PH >= 4:
            nc.gpsimd.collective_compute(
                "AllGather", mybir.AluOpType.bypass,
                replica_groups=[list(range(N_CORES))],
                ins=[z1loc[:]], outs=[z1src_g[:]])

            edge_pass(1)

    nc.compile()
    return nc


# ---------------------------------------------------------------- entry

def kernel(**inputs):
    cfg = CFG(N=inputs["x"].shape[0], E=inputs["edge_index"].shape[1],
              IN=inputs["x"].shape[1], TD=inputs["time_w"].shape[0],
              H=np.asarray(inputs["att_src0"]).shape[0],
              C0=np.asarray(inputs["att_src0"]).shape[1],
              C1=np.asarray(inputs["att_src1"]).shape[1])
    in_maps, meta = _host_prep(cfg, **{k: np.asarray(v) for k, v in inputs.items()})
    nc = _build_program(cfg, meta)
    res = run_bass_kernel_spmd(nc, in_maps, list(range(N_CORES)))
    blocks = [res.results[k]["out"] for k in range(N_CORES)]
    return np.concatenate(blocks, axis=0)[:cfg.N].astype(np.float32)


# revision 20
# speedup vs baseline: 1.5541x; 1.0860x over previous
"""Trainium2 Bass kernel for nn_GAT_Simple (2-layer GATConv with time-encoded
edge attrs, 50k nodes / 800k edges, 8 NeuronCores).

Sharding: nodes partitioned into 8 blocks; core k owns destination block k.
Per-core tables are node-ROTATED so each core's own block sits at rows [0,NB)
of its local tables, keeping the SPMD program core-independent.

Phases per core:
  Z0       - full (rotated) projected tables z0src = bf16 [x@W0 | alpha_src
             terms], z0dst = fp32 alpha_dst terms for the local block only.
  prephase - per-edge time-encoder attention scalars se = |cos(ts*w+b)| @ M
             for BOTH layers into a resident SBUF buffer (keeps the Sin ACT
             table set out of the edge loop, which uses the Exp set).
  L0 edges - per 128-dst-node window: per-tile 128-row indirect gathers of
             bf16 source rows; alpha -> leaky-relu -> exp; ad[dst] expanded
             from a dense per-window vector via a transposed one-hot matmul
             accumulated in PSUM; weighted segment-sum + softmax denominator
             via a one-hot matmul per tile accumulated in PSUM (normalize
             after aggregation => single pass over edges). Window flush adds
             self-loops densely, normalizes, and fuses the layer-1 projection.
  AllGather of per-block bf16 z1 tables across the 8 cores.
  L1 edges - same structure on the gathered z1 tables; adds bias1; writes the
             per-core fp32 output block.
"""

import math
import os
from contextlib import ExitStack

import numpy as np

try:
    import ml_dtypes  # noqa: F401  (registers bfloat16 with numpy)
    _BF16 = np.dtype("bfloat16")
except ImportError:
    _BF16 = None

import concourse.bass as bass
import concourse.bacc as bacc
import concourse.tile as tile
from concourse import mybir
from concourse.bass_utils import run_bass_kernel_spmd

FP = mybir.dt.float32
BF = mybir.dt.bfloat16
I32 = mybir.dt.int32

N_CORES = 8
SLOPE = 0.2


class CFG:
    def __init__(self, N=50000, E=800000, IN=128, TD=64, H=4, C0=64, C1=16):
        self.N, self.E, self.IN, self.TD = N, E, IN, TD
        self.H, self.C0, self.C1 = H, C0, C1
        self.HC0, self.HC1 = H * C0, H * C1              # 256, 64
        self.D0 = self.HC0 + 4                            # z0 | as0  -> 260
        self.D1 = self.HC1 + 4                            # z1 | as1  -> 68
        self.NP = ((N + N_CORES * 128 - 1) // (N_CORES * 128)) * (N_CORES * 128)
        self.NB = self.NP // N_CORES
        self.W = self.NB // 128


# ---------------------------------------------------------------- host prep

def _host_prep(cfg, x, edge_index, timestamps, time_w, time_b,
               W0, att_src0, att_dst0, lin_edge0, att_edge0, bias0,
               W1, att_src1, att_dst1, lin_edge1, att_edge1, bias1):
    N, NP, NB, W = cfg.N, cfg.NP, cfg.NB, cfg.W
    H, C0, C1, TD, IN = cfg.H, cfg.C0, cfg.C1, cfg.TD, cfg.IN
    HC0, HC1 = cfg.HC0, cfg.HC1

    f32 = np.float32
    bf = _BF16 if int(os.environ.get("KBF16", "1")) else f32
    x = np.asarray(x, f32)
    src = np.asarray(edge_index[0], np.int64)
    dst = np.asarray(edge_index[1], np.int64)
    ts = np.asarray(timestamps, f32)

    def blockdiag(att, C):  # [H, C] -> [H*C, H]
        out = np.zeros((H * C, H), f32)
        for h in range(H):
            out[h * C:(h + 1) * C, h] = att[h]
        return out

    W0 = np.asarray(W0, f32); W1 = np.asarray(W1, f32)
    W0ext = np.concatenate(
        [W0, W0 @ blockdiag(np.asarray(att_src0, f32), C0),
         W0 @ blockdiag(np.asarray(att_dst0, f32), C0)], axis=1)  # [IN, HC0+8]
    W1ext = np.concatenate(
        [W1, W1 @ blockdiag(np.asarray(att_src1, f32), C1),
         W1 @ blockdiag(np.asarray(att_dst1, f32), C1)], axis=1)  # [HC0, HC1+8]
    c1ext = np.asarray(bias0, f32) @ W1ext                         # [HC1+8]

    M0c = np.einsum("thc,hc->th",
                    np.asarray(lin_edge0, f32).reshape(TD, H, C0),
                    np.asarray(att_edge0, f32))
    M1c = np.einsum("thc,hc->th",
                    np.asarray(lin_edge1, f32).reshape(TD, H, C1),
                    np.asarray(att_edge1, f32))
    M01 = np.concatenate([M0c, M1c], axis=1).astype(f32)           # [TD, 2H]
    M01x2 = np.concatenate([M01, M01], axis=0)                     # [2TD, 2H]
    sl0 = M0c.sum(axis=0).astype(f32)
    sl1 = M1c.sum(axis=0).astype(f32)

    # |cos(ts*w+b)| = |sin(2*pi*r)|, r = t1 - round(t1),
    # t1 = ts*(w/2pi) + (b+pi/2)/(2pi)
    w2pi = (np.asarray(time_w, f32) / (2 * np.pi)).astype(f32)
    b2pi = ((np.asarray(time_b, f32) + np.pi / 2) / (2 * np.pi)).astype(f32)
    efsc = np.tile(w2pi, 2).reshape(128, 1)
    efbi = np.tile(b2pi, 2).reshape(128, 1)

    ones2 = np.zeros((2, 128), f32)
    ones2[0, :64] = 1.0
    ones2[1, 64:] = 1.0
    ones1 = np.ones((1, 128), bf)

    iota_row = np.broadcast_to(np.arange(128, dtype=f32), (128, 128)).copy()
    iota_col = np.broadcast_to(np.arange(128, dtype=f32)[:, None],
                               (128, 128)).copy()
    identity_bf = np.eye(128).astype(bf)

    xpadT = np.zeros((IN, NP), f32)
    xpadT[:, :N] = x.T

    blk = dst // NB
    per_core = []
    for k in range(N_CORES):
        m = blk == k
        s_k, d_k, t_k = src[m], dst[m], ts[m]
        order = np.argsort(d_k, kind="stable")
        per_core.append((s_k[order], d_k[order], t_k[order]))

    cnt = np.zeros((N_CORES, W), np.int64)
    for k in range(N_CORES):
        d_k = per_core[k][1] - k * NB
        cnt[k] = np.bincount(d_k // 128, minlength=W)
    Tw = np.maximum(1, (cnt.max(axis=0) + 127) // 128).astype(np.int64)
    CT = int(Tw.sum())
    CTpre = ((CT + 7) // 8) * 8
    colbase = np.zeros(W + 1, np.int64)
    colbase[1:] = np.cumsum(Tw)

    consts = {
        "W0ext": W0ext,
        "W1extA": W1ext[:128].astype(bf), "W1extB": W1ext[128:].astype(bf),
        "M01": M01x2, "efsc": efsc, "efbi": efbi, "ones2": ones2,
        "ones1": ones1,
        "iota": iota_row, "iotac": iota_col, "ident": identity_bf,
        "sl0": np.broadcast_to(sl0, (128, H)).copy(),
        "sl1": np.broadcast_to(sl1, (128, H)).copy(),
        "bias1": np.broadcast_to(np.asarray(bias1, f32), (128, HC1)).copy(),
        "c1ext": np.broadcast_to(c1ext, (128, HC1 + 8)).copy(),
    }

    in_maps = []
    for k in range(N_CORES):
        s_k, d_k, t_k = per_core[k]
        esrc0 = np.zeros((128, CT), np.int32)
        esrc1 = np.zeros((128, CT), np.int32)
        eslot = np.zeros((128, CT), f32)
        eslotT = np.zeros(CT * 128, bf)
        evalid = np.zeros((128, CT * H), f32)
        ets = np.zeros(CTpre * 128, f32)

        pos = 0
        for w in range(W):
            n = int(cnt[k, w])
            sw = s_k[pos:pos + n]; dw = d_k[pos:pos + n]; tw = t_k[pos:pos + n]
            pos += n
            G = int(Tw[w]); c0 = int(colbase[w])
            sl = (dw - k * NB - w * 128).astype(f32)
            for g in range(G):
                lo = g * 128
                m = min((g + 1) * 128, n) - lo
                if m <= 0:
                    continue
                c = c0 + g
                esrc0[:m, c] = ((sw[lo:lo + m] - k * NB) % NP).astype(np.int32)
                esrc1[:m, c] = sw[lo:lo + m].astype(np.int32)
                eslot[:m, c] = sl[lo:lo + m]
                eslotT[c * 128:c * 128 + m] = sl[lo:lo + m].astype(bf)
                evalid[:m, c * H:(c + 1) * H] = 1.0
                ets[c * 128:c * 128 + m] = tw[lo:lo + m]
        im = {
            "xT": np.roll(xpadT, -k * NB, axis=1),
            "esrc0": esrc0, "esrc1": esrc1,
            "eslot": eslot, "eslotT": eslotT, "evalid": evalid, "ets": ets,
        }
        im.update(consts)
        in_maps.append(im)

    meta = {"Tw": Tw, "colbase": colbase, "CT": CT, "CTpre": CTpre}
    return in_maps, meta


# ---------------------------------------------------------------- program

def _build_program(cfg, meta):
    NP, NB, W = cfg.NP, cfg.NB, cfg.W
    H, TD, IN = cfg.H, cfg.TD, cfg.IN
    HC0, HC1, D0, D1 = cfg.HC0, cfg.HC1, cfg.D0, cfg.D1
    Tw, colbase, CT, CTpre = meta["Tw"], meta["colbase"], meta["CT"], meta["CTpre"]
    Gmax = int(max(Tw))

    TB = BF if int(os.environ.get("KBF16", "1")) else FP
    nc = bacc.Bacc("TRN2", target_bir_lowering=False, debug=False,
                   num_devices=N_CORES)
    P = nc.declare_dram_parameter

    xT = P("xT", [IN, NP], FP, isOutput=False)
    esrc0 = P("esrc0", [128, CT], I32, isOutput=False)
    esrc1 = P("esrc1", [128, CT], I32, isOutput=False)
    eslot = P("eslot", [128, CT], FP, isOutput=False)
    eslotT = P("eslotT", [CT * 128], TB, isOutput=False)
    evalid = P("evalid", [128, CT * H], FP, isOutput=False)
    ets = P("ets", [CTpre * 128], FP, isOutput=False)
    W0ext_d = P("W0ext", [IN, HC0 + 8], FP, isOutput=False)
    W1A_d = P("W1extA", [128, HC1 + 8], TB, isOutput=False)
    W1B_d = P("W1extB", [128, HC1 + 8], TB, isOutput=False)
    M01_d = P("M01", [2 * TD, 2 * H], FP, isOutput=False)
    efsc_d = P("efsc", [128, 1], FP, isOutput=False)
    efbi_d = P("efbi", [128, 1], FP, isOutput=False)
    ones2_d = P("ones2", [2, 128], FP, isOutput=False)
    ones1_d = P("ones1", [1, 128], TB, isOutput=False)
    iota_d = P("iota", [128, 128], FP, isOutput=False)
    iotac_d = P("iotac", [128, 128], FP, isOutput=False)
    ident_d = P("ident", [128, 128], TB, isOutput=False)
    sl0_d = P("sl0", [128, H], FP, isOutput=False)
    sl1_d = P("sl1", [128, H], FP, isOutput=False)
    bias1_d = P("bias1", [128, HC1], FP, isOutput=False)
    c1ext_d = P("c1ext", [128, HC1 + 8], FP, isOutput=False)

    out_d = P("out", [NB, HC1], FP, isOutput=True)

    z0src = nc.dram_tensor("z0src", [NP, D0], TB)
    z0dst = nc.dram_tensor("z0dst", [NB, H], FP)
    z1loc = nc.dram_tensor("z1loc", [NB, D1], TB)
    z1dloc = nc.dram_tensor("z1dloc", [NB, H], FP)
    z1src_g = nc.dram_tensor("z1src_g", [NP, D1], TB, addr_space="Shared")

    AF = mybir.ActivationFunctionType
    OP = mybir.AluOpType
    TWO_PI = 2 * math.pi
    eslotT_v = eslotT[:].rearrange("(c p) -> c p", p=128)

    with ExitStack() as ctx:
        tc = ctx.enter_context(tile.TileContext(nc))
        cpool = ctx.enter_context(tc.tile_pool(name="consts", bufs=1))
        sepool = ctx.enter_context(tc.tile_pool(name="seall", bufs=1))
        sbuf = ctx.enter_context(tc.tile_pool(name="sbuf", bufs=3))
        gpool = ctx.enter_context(tc.tile_pool(name="gather", bufs=3))
        rpool = ctx.enter_context(tc.tile_pool(name="rhs", bufs=3))
        mpool = ctx.enter_context(tc.tile_pool(name="onehot", bufs=2))
        mtpool = ctx.enter_context(tc.tile_pool(name="onehot_t", bufs=2))
        fpool = ctx.enter_context(tc.tile_pool(name="flush", bufs=2))
        ppre = ctx.enter_context(tc.tile_pool(name="ppre", bufs=2, space="PSUM"))
        pse = ctx.enter_context(tc.tile_pool(name="pse", bufs=2, space="PSUM"))
        pseg = ctx.enter_context(tc.tile_pool(name="pseg", bufs=2, space="PSUM"))
        ptr = ctx.enter_context(tc.tile_pool(name="ptr", bufs=1, space="PSUM"))
        pz1 = ctx.enter_context(tc.tile_pool(name="pz1", bufs=1, space="PSUM"))

        def cload(dram, shape, tag, dtype=FP):
            t = cpool.tile(shape, dtype, tag=tag)
            nc.sync.dma_start(out=t[:], in_=dram[:])
            return t

        W0ext_s = cload(W0ext_d, [IN, HC0 + 8], "w0ext")
        W1A_s = cload(W1A_d, [128, HC1 + 8], "w1a", TB)
        W1B_s = cload(W1B_d, [128, HC1 + 8], "w1b", TB)
        M01_s = cload(M01_d, [2 * TD, 2 * H], "m01")
        efsc_s = cload(efsc_d, [128, 1], "efsc")
        efbi_s = cload(efbi_d, [128, 1], "efbi")
        ones2_s = cload(ones2_d, [2, 128], "ones2")
        ones1_s = cload(ones1_d, [1, 128], "ones1", TB)
        iota_s = cload(iota_d, [128, 128], "iota")
        iotac_s = cload(iotac_d, [128, 128], "iotac")
        ident_s = cload(ident_d, [128, 128], "ident", TB)
        sl0_s = cload(sl0_d, [128, H], "sl0")
        sl1_s = cload(sl1_d, [128, H], "sl1")
        bias1_s = cload(bias1_d, [128, HC1], "bias1")
        c1ext_s = cload(c1ext_d, [128, HC1 + 8], "c1ext")

        se_all = sepool.tile([128, CTpre * 2 * H], FP)

        PH = int(os.environ.get("KPHASES", "4"))
        KDEBUG = int(os.environ.get("KDEBUG", "0"))

        # ---- Phase Z0: rotated tables
        for t in range(NP // 128):
            xt = sbuf.tile([IN, 128], FP, tag="xt")
            nc.sync.dma_start(out=xt[:], in_=xT[:, t * 128:(t + 1) * 128])
            ps = pseg.tile([128, HC0 + 8], FP, tag="segp")
            nc.tensor.matmul(out=ps[:], lhsT=xt[:], rhs=W0ext_s[:],
                             start=True, stop=True)
            zt = sbuf.tile([128, D0], TB, tag="zt")
            nc.vector.tensor_copy(out=zt[:], in_=ps[:, :D0])
            nc.sync.dma_start(out=z0src[t * 128:(t + 1) * 128, :], in_=zt[:])
            if t < W:
                zt4 = sbuf.tile([128, H], FP, tag="zt4")
                nc.vector.tensor_copy(out=zt4[:], in_=ps[:, D0:D0 + H])
                nc.sync.dma_start(out=z0dst[t * 128:(t + 1) * 128, :],
                                  in_=zt4[:])

        # ---- Prephase: se_all
        ets_v = ets[:].rearrange("(a b) -> a b", b=512)
        for c in (range(0, CTpre, 8) if PH >= 2 else []):
            ts2 = sbuf.tile([2, 512], FP, tag="ts2")
            row = c * 128 // 512
            nc.sync.dma_start(out=ts2[:], in_=ets_v[row:row + 2, :])
            rp = ppre.tile([128, 512], FP, tag="prep")
            nc.tensor.matmul(out=rp[:], lhsT=ones2_s[:], rhs=ts2[:],
                             start=True, stop=True)
            t1 = sbuf.tile([128, 512], FP, tag="t1")
            nc.scalar.activation(out=t1[:], in_=rp[:], func=AF.Identity,
                                 bias=efbi_s[:], scale=efsc_s[:])
            k32 = sbuf.tile([128, 512], I32, tag="k32")
            nc.vector.tensor_copy(out=k32[:], in_=t1[:])
            kf = sbuf.tile([128, 512], FP, tag="kf")
            nc.vector.tensor_copy(out=kf[:], in_=k32[:])
            nc.vector.tensor_tensor(out=t1[:], in0=t1[:], in1=kf[:],
                                    op=OP.subtract)
            nc.scalar.activation(out=t1[:], in_=t1[:], func=AF.Sin,
                                 scale=TWO_PI)
            nc.scalar.activation(out=t1[:], in_=t1[:], func=AF.Abs)
            for b in range(8):
                half, coff = (b // 4) * 64, (b % 4) * 128
                sp = ppre.tile([128, 2 * H], FP, tag="prep")
                nc.tensor.matmul(out=sp[:],
                                 lhsT=t1[half:half + 64, coff:coff + 128],
                                 rhs=M01_s[half:half + 64, :],
                                 start=True, stop=True)
                nc.vector.tensor_copy(
                    out=se_all[:, (c + b) * 2 * H:(c + b + 1) * 2 * H],
                    in_=sp[:])

        # ---- edge pass
        def edge_pass(layer):
            if layer == 0:
                table_s, dense_s, dense_d = z0src, z0src, z0dst
                esrc = esrc0
                DS, HC, seoff = D0, HC0, 0
            else:
                table_s, dense_s, dense_d = z1src_g, z1loc, z1dloc
                esrc = esrc1
                DS, HC, seoff = D1, HC1, H
            Cc = HC // H
            for w in range(W):
                G = int(Tw[w]); c0 = int(colbase[w])
                r0 = w * 128
                isrc = sbuf.tile([128, G], I32, tag="isrc")
                nc.sync.dma_start(out=isrc[:], in_=esrc[:, c0:c0 + G])
                slotf = sbuf.tile([128, G], FP, tag="slotf")
                nc.sync.dma_start(out=slotf[:], in_=eslot[:, c0:c0 + G])
                evw = sbuf.tile([128, G * H], FP, tag="evw")
                nc.sync.dma_start(out=evw[:],
                                  in_=evalid[:, c0 * H:(c0 + G) * H])
                # dense own-window rows (self loops + ad_w)
                zw = fpool.tile([128, DS], TB, tag=f"zw{layer}")
                nc.sync.dma_start(out=zw[:], in_=dense_s[r0:r0 + 128, :])
                adw = fpool.tile([128, H], FP, tag="adw")
                nc.sync.dma_start(out=adw[:], in_=dense_d[r0:r0 + 128, :])
                adw_bf = fpool.tile([128, H], TB, tag="adwbf")
                nc.vector.tensor_copy(out=adw_bf[:], in_=adw[:])

                zg = gpool.tile([128, G * DS], TB, tag=f"zg{layer}")
                for g in range(G):
                    nc.gpsimd.indirect_dma_start(
                        out=zg[:, g * DS:(g + 1) * DS], out_offset=None,
                        in_=table_s[:],
                        in_offset=bass.IndirectOffsetOnAxis(
                            ap=isrc[:, g:g + 1], axis=0))
                # one-hot M_all[e, (g, slot)] in one DVE op
                M_all = mpool.tile([128, G * 128], TB, tag="M")
                nc.vector.tensor_tensor(
                    out=M_all[:].rearrange("p (g s) -> p g s", s=128),
                    in0=slotf[:].rearrange("p (g o) -> p g o",
                                           o=1).to_broadcast([128, G, 128]),
                    in1=iota_s[:].rearrange("p (o s) -> p o s",
                                            o=1).to_broadcast([128, G, 128]),
                    op=OP.is_equal)
                # transposed one-hots: tiny row load + ones-matmul
                # replication into PSUM chunks + is_eq
                srow = sbuf.tile([1, G * 128], TB, tag="srow")
                nc.sync.dma_start(out=srow[:],
                                  in_=eslotT_v[c0:c0 + G, :].rearrange(
                                      "g e -> (g e)")[None, :])
                Mt_all = mtpool.tile([128, G * 128], TB, tag="Mt")
                for q0 in range(0, G * 128, 512):
                    qn = min(512, G * 128 - q0)
                    rps = ppre.tile([128, 512], FP, tag="prep")
                    nc.tensor.matmul(out=rps[:, :qn], lhsT=ones1_s[:],
                                     rhs=srow[:, q0:q0 + qn],
                                     start=True, stop=True)
                    nc.vector.tensor_tensor(
                        out=Mt_all[:, q0:q0 + qn].rearrange(
                            "p (a e) -> p a e", e=128),
                        in0=rps[:, :qn].rearrange("p (a e) -> p a e", e=128),
                        in1=iotac_s[:].rearrange(
                            "p (o e) -> p o e", o=1).to_broadcast(
                                [128, qn // 128, 128]),
                        op=OP.is_equal)
                adps = pse.tile([128, G * H], FP, tag="adps")
                for g in range(G):
                    nc.tensor.matmul(out=adps[:, g * H:(g + 1) * H],
                                     lhsT=Mt_all[:, g * 128:(g + 1) * 128],
                                     rhs=adw_bf[:], start=True, stop=True)

                zg_v = zg[:].rearrange("p (g d) -> p g d", d=DS)
                se_v = se_all[:, c0 * 2 * H:(c0 + G) * 2 * H].rearrange(
                    "p (g e) -> p g e", e=2 * H)

                # alpha = se + as[src] + ad[dst]; lrelu; exp; mask padding
                alpha = sbuf.tile([128, G * H], FP, tag="alpha")
                al_v = alpha[:].rearrange("p (g h) -> p g h", h=H)
                nc.vector.tensor_tensor(
                    out=al_v, in0=se_v[:, :, seoff:seoff + H],
                    in1=zg_v[:, :, HC:HC + H], op=OP.add)
                nc.vector.tensor_tensor(
                    out=alpha[:], in0=alpha[:], in1=adps[:], op=OP.add)
                scaled = sbuf.tile([128, G * H], FP, tag="scaled")
                nc.vector.tensor_scalar(out=scaled[:], in0=alpha[:],
                                        scalar1=SLOPE, scalar2=None,
                                        op0=OP.mult)
                nc.vector.tensor_tensor(out=alpha[:], in0=alpha[:],
                                        in1=scaled[:], op=OP.max)
                expw = sbuf.tile([128, G * H], FP, tag="expw")
                nc.scalar.activation(out=expw[:], in_=alpha[:], func=AF.Exp)
                nc.vector.tensor_tensor(out=expw[:], in0=expw[:],
                                        in1=evw[:], op=OP.mult)
                ex_v = expw[:].rearrange("p (g h) -> p g h", h=H)

                ps = pseg.tile([128, HC + H], FP, tag="segp")
                for g in range(G):
                    rhs = rpool.tile([128, HC + H], TB, tag=f"rhs{layer}")
                    nc.vector.tensor_tensor(
                        out=rhs[:, :HC].rearrange("p (h c) -> p h c", c=Cc),
                        in0=zg_v[:, g, :HC].rearrange("p (h c) -> p h c",
                                                      c=Cc),
                        in1=ex_v[:, g, :].rearrange(
                            "p (h o) -> p h o", o=1).to_broadcast(
                                [128, H, Cc]),
                        op=OP.mult)
                    nc.scalar.copy(out=rhs[:, HC:HC + H],
                                   in_=expw[:, g * H:(g + 1) * H])
                    nc.tensor.matmul(out=ps[:],
                                     lhsT=M_all[:, g * 128:(g + 1) * 128],
                                     rhs=rhs[:],
                                     start=(g == 0), stop=(g == G - 1))

                # ---- flush
                sl_s = sl0_s if layer == 0 else sl1_s
                asel = fpool.tile([128, H], FP, tag="asel")
                nc.vector.tensor_tensor(out=asel[:], in0=zw[:, HC:HC + H],
                                        in1=adw[:], op=OP.add)
                nc.vector.tensor_tensor(out=asel[:], in0=asel[:],
                                        in1=sl_s[:], op=OP.add)
                ssc = fpool.tile([128, H], FP, tag="ssc")
                nc.vector.tensor_scalar(out=ssc[:], in0=asel[:], scalar1=SLOPE,
                                        scalar2=None, op0=OP.mult)
                nc.vector.tensor_tensor(out=asel[:], in0=asel[:], in1=ssc[:],
                                        op=OP.max)
                nc.scalar.activation(out=asel[:], in_=asel[:], func=AF.Exp)
                den = fpool.tile([128, H], FP, tag="den")
                nc.vector.tensor_tensor(out=den[:], in0=ps[:, HC:HC + H],
                                        in1=asel[:], op=OP.add)
                rec = fpool.tile([128, H], FP, tag="rec")
                nc.vector.reciprocal(out=rec[:], in_=den[:])
                o0 = fpool.tile([128, HC], FP, tag=f"o0{layer}")
                for h in range(H):
                    hs = slice(h * Cc, (h + 1) * Cc)
                    nc.vector.tensor_scalar(
                        out=o0[:, hs], in0=zw[:, hs],
                        scalar1=asel[:, h:h + 1], scalar2=None, op0=OP.mult)
                    nc.vector.tensor_tensor(out=o0[:, hs], in0=o0[:, hs],
                                            in1=ps[:, hs], op=OP.add)
                    nc.vector.tensor_scalar(
                        out=o0[:, hs], in0=o0[:, hs],
                        scalar1=rec[:, h:h + 1], scalar2=None, op0=OP.mult)

                if layer == 0:
                    o0b = fpool.tile([128, HC], TB, tag="o0b")
                    nc.vector.tensor_copy(out=o0b[:], in_=o0[:])
                    z1p = pz1.tile([128, HC1 + 8], FP, tag="z1p")
                    for half, Wh in ((0, W1A_s), (1, W1B_s)):
                        tp = ptr.tile([128, 128], TB, tag="tp")
                        nc.tensor.transpose(
                            out=tp[:],
                            in_=o0b[:, half * 128:(half + 1) * 128],
                            identity=ident_s[:])
                        oT = fpool.tile([128, 128], TB, tag="oT")
                        nc.vector.tensor_copy(out=oT[:], in_=tp[:])
                        nc.tensor.matmul(out=z1p[:], lhsT=oT[:], rhs=Wh[:],
                                         start=(half == 0), stop=(half == 1))
                    z1e = fpool.tile([128, D1], TB, tag="z1e")
                    nc.vector.tensor_tensor(out=z1e[:], in0=z1p[:, :D1],
                                            in1=c1ext_s[:, :D1], op=OP.add)
                    nc.sync.dma_start(out=z1loc[r0:r0 + 128, :], in_=z1e[:])
                    z1e4 = fpool.tile([128, H], FP, tag="z1e4")
                    nc.vector.tensor_tensor(out=z1e4[:],
                                            in0=z1p[:, D1:D1 + H],
                                            in1=c1ext_s[:, D1:D1 + H],
                                            op=OP.add)
                    nc.sync.dma_start(out=z1dloc[r0:r0 + 128, :], in_=z1e4[:])
                else:
                    nc.vector.tensor_tensor(out=o0[:], in0=o0[:],
                                            in1=bias1_s[:], op=OP.add)
                    nc.sync.dma_start(out=out_d[r0:r0 + 128, :], in_=o0[:])

        if PH >= 3:
            edge_pass(0)

        if PH >= 4:
            nc.gpsimd.collective_compute(
                "AllGather", mybir.AluOpType.bypass,
                replica_groups=[list(range(N_CORES))],
                ins=[z1loc[:]], outs=[z1src_g[:]])

            edge_pass(1)

    nc.compile()
    return nc


# ---------------------------------------------------------------- entry

def kernel(**inputs):
    cfg = CFG(N=inputs["x"].shape[0], E=inputs["edge_index"].shape[1],
              IN=inputs["x"].shape[1], TD=inputs["time_w"].shape[0],
              H=np.asarray(inputs["att_src0"]).shape[0],
              C0=np.asarray(inputs["att_src0"]).shape[1],
              C1=np.asarray(inputs["att_src1"]).shape[1])
    in_maps, meta = _host_prep(cfg, **{k: np.asarray(v) for k, v in inputs.items()})
    nc = _build_program(cfg, meta)
    res = run_bass_kernel_spmd(nc, in_maps, list(range(N_CORES)))
    blocks = [res.results[k]["out"] for k in range(N_CORES)]
    return np.concatenate(blocks, axis=0)[:cfg.N].astype(np.float32)
